# revision 11
# baseline (speedup 1.0000x reference)
"""LurieNet-k Trainium2 kernel (lag-4 paired recurrence, fp16 operands).

Computes, from the raw parametrization tensors, the matrices
  C = UC @ SC @ VC^T,  B = UB @ SB @ VB^T,
  A = 0.5*UA @ SA @ UA^T + 0.5*YA  (SA = -(alpha_upp*I + GA))
entirely on device (matrix exponentials of skew matrices via
scaling-and-squaring Taylor), then runs the 511-step recurrence
  u_t = tanh(C x_t + by);  x_{t+1} = x_t + 0.01*(A x_t + B u_t + bx)
on a (128, 64) state shard per NeuronCore (batch data-parallel over
the 8 cores).

Structure: the naive step is a serial tanh->matmul->tanh round trip
(~860ns on TRN2: ACT access latency + 2 sem hops + PE). Because the
tanh self-coupling Q = 0.01*C*B has tiny norm (~3e-4), the recurrence
is re-expanded to an (almost) exact LAG-4 form: every quantity at
step t is computed from state/tanh values at steps t-4/t-3 (and t-8
/t-7 for a first-order staleness extrapolation of the x-chain's u
terms, which kills the dominant scheme error: measured 1.2e-2 plain
-> 3.2e-4 extrapolated, fp16). All matmul inputs are then >= 2
pair-iterations old, so nothing serializes, and steps are processed
in PAIRS:
  - one 128-wide matmul per weight per pair (halves LDWEIGHTS, the
    PE throughput limit)
  - one 128-wide tanh per pair (halves ACT's per-instruction access
    latency tax)
  - PSUM evacuation split across ACT (Copy w/ bias for delta_t),
    DVE (pair-sum + the two fp16 state writes), and GPSIMD (fp32
    pair carry, SBUF-only since GPSIMD cannot read PSUM).
Weights/states/tanh values are fp16 (same PE speed as bf16, 8x finer
rounding); the fp32 carry keeps the state exact (all partial sums
fp32). Output is written fp16 time-major [n, t, b] straight from the
state buffer (no on-device transpose) and transposed to (b, t, n)
fp32 on the host during unsharding; the t=0 plane is restored
exactly from X0. Measured end-to-end rel err ~4e-4 (budget 2e-2).
"""

import sys

for _p in ("/opt/trn_rl_repo",):
    if _p not in sys.path:
        sys.path.insert(0, _p)

import numpy as np

import concourse.bass as bass
import concourse.mybir as mybir
import concourse.tile as tile
from concourse import bacc
from concourse import bass_isa
from concourse.bass import ds
from concourse.bass_utils import run_bass_kernel_spmd
from concourse.masks import make_identity, make_upper_triangular

F32 = mybir.dt.float32
F32R = mybir.dt.float32r
FP16 = mybir.dt.float16
ALU = mybir.AluOpType
ACTF = mybir.ActivationFunctionType
AXIS = mybir.AxisListType

N = 128          # state dim
TMAX = 512       # time steps (including t=0)
BS = 512         # global batch
NCORES = 8
BSH = BS // NCORES   # 64 batch columns per core
STEP = 0.01
KTOP = 4

EXPM_SCAL = 3    # expm scaling: X = S / 2**EXPM_SCAL, then 3 squarings
EXPM_TERMS = 4   # Taylor terms in the Horner evaluation

PARAM_NAMES = [
    "ZA_Y", "ZA_U", "ZA_G", "ZB_U", "ZB_V", "ZB_S", "ZC_U", "ZC_V", "ZC_S",
]


def build_program(tmax=TMAX, tc_chunk=64, mdt=FP16):
    """Build the single-NeuronCore Bass program (run SPMD on all 8 cores)."""
    assert tmax % tc_chunk == 0 and tc_chunk % 2 == 0
    nc = bacc.Bacc(
        "TRN2",
        target_bir_lowering=False,
        debug=False,
        enable_asserts=False,
        num_devices=NCORES,
    )

    x0 = nc.dram_tensor("x0", [N, BSH], F32, kind="ExternalInput")
    zs = {
        name: nc.dram_tensor(name, [N, N], F32, kind="ExternalInput")
        for name in PARAM_NAMES
    }
    bx_d = nc.dram_tensor("bx", [N, 1], F32, kind="ExternalInput")
    by_d = nc.dram_tensor("by", [N, 1], F32, kind="ExternalInput")
    # time-major fp16 output: out[n, t*BSH + b]; host transposes to (b,t,n)
    out = nc.dram_tensor("out", [N, tmax * BSH], FP16, kind="ExternalOutput")

    with tile.TileContext(nc) as tc:
        with tc.tile_pool(name="const", bufs=1) as constp:
            ident = constp.tile([N, N], F32, tag="ident")
            make_identity(nc, ident[:])
            masku = constp.tile([N, N], F32, tag="masku")
            make_upper_triangular(nc, masku[:], val=1.0, diag=False)
            ident_r32 = constp.tile([N, N], F32R, tag="ident_r32")
            nc.vector.tensor_copy(ident_r32[:], ident[:])

            by_c = constp.tile([N, 1], F32, tag="by")
            nc.sync.dma_start(out=by_c[:], in_=by_d[:])
            bx_c = constp.tile([N, 1], F32, tag="bxraw")
            nc.sync.dma_start(out=bx_c[:], in_=bx_d[:])
            bxp_c = constp.tile([N, 1], F32, tag="bxp")
            nc.vector.tensor_scalar_mul(bxp_c[:], bx_c[:], STEP)
            x0_c = constp.tile([N, BSH], F32, tag="x0c")
            nc.sync.dma_start(out=x0_c[:], in_=x0[:])

            # ------- runtime weights (transposed, fp16) -------
            P4Tm = constp.tile([N, N], mdt, tag="P4Tm")    # (C At^4)^T
            Q4Tm = constp.tile([N, N], mdt, tag="Q4Tm")    # (C S3 G)^T
            A4Tm = constp.tile([N, N], mdt, tag="A4Tm")    # (0.01A At^3)^T
            GATm = constp.tile([N, N], mdt, tag="GATm")    # extrap u_{t-4} w
            GBTm = constp.tile([N, N], mdt, tag="GBTm")    # extrap u_{t-8} w
            G4Tm = constp.tile([N, N], mdt, tag="G4Tm")    # plain (boot pairs)
            A01Tm = constp.tile([N, N], mdt, tag="A01Tm")  # (0.01 A)^T (boot)
            BpTm = constp.tile([N, N], mdt, tag="BpTm")    # (0.01 B)^T (boot)
            CTf32 = constp.tile([N, N], F32, tag="CTf32")  # C^T fp32 (boot)
            r4_c = constp.tile([N, 1], F32, tag="r4c")     # C S3 c + by
            cc_c = constp.tile([N, 1], F32, tag="ccc")     # delta const

            with (
                tc.tile_pool(name="zbuf", bufs=1) as zp,
                tc.tile_pool(name="work", bufs=2) as wp,
                tc.tile_pool(name="eres", bufs=1) as ep,
                tc.tile_pool(name="small", bufs=1) as sp,
                tc.tile_pool(name="pss", bufs=4, space="PSUM") as psp,
            ):
                zt = {}
                for name in PARAM_NAMES:
                    zt[name] = zp.tile([N, N], F32, tag=name, name=f"z_{name}")
                    nc.sync.dma_start(out=zt[name][:], in_=zs[name][:])

                def expm_batch(specs):
                    """Interleaved expm(skew(Z))^T for all matrices at once.

                    Maintains the (T, T^T) pair through Horner + squaring so
                    no PE transposes are needed: with negX = X^T = -X,
                      X @ T     = matmul(lhsT=negX, rhs=T)
                      T^T @ X^T = matmul(lhsT=T,    rhs=negX)
                    """
                    scal = 1.0 / (2.0 ** EXPM_SCAL)
                    negx = {}
                    t_cur = {}
                    tt_cur = {}
                    for z_tile, tag in specs:
                        us = wp.tile([N, N], F32R, tag="us_r", name=f"us_{tag}")
                        nc.vector.scalar_tensor_tensor(
                            us[:], z_tile[:], scal, masku[:],
                            op0=ALU.mult, op1=ALU.mult,
                        )
                        pst = psp.tile([N, N], F32R, tag="ps", bufs=8,
                                       name=f"pst_{tag}")
                        nc.tensor.transpose(pst[:], us[:], ident_r32[:])
                        nx = wp.tile([N, N], F32R, tag=f"negx_{tag}", bufs=1,
                                     name=f"negx_{tag}")
                        nc.vector.scalar_tensor_tensor(
                            nx[:], pst[:], 1.0, us[:],
                            op0=ALU.mult, op1=ALU.subtract,
                        )
                        negx[tag] = nx
                        t_cur[tag] = ident_r32
                        tt_cur[tag] = ident_r32
                    for j in range(EXPM_TERMS, 0, -1):
                        for _, tag in specs:
                            psa = psp.tile([N, N], F32, tag="ps", bufs=8)
                            nc.tensor.matmul(
                                psa[:], negx[tag][:], t_cur[tag][:],
                                start=True, stop=True,
                            )
                            t_new = wp.tile([N, N], F32R, tag=f"T_{tag}",
                                            bufs=2, name=f"T_{tag}")
                            nc.vector.scalar_tensor_tensor(
                                t_new[:], psa[:], 1.0 / j, ident_r32[:],
                                op0=ALU.mult, op1=ALU.add,
                            )
                            t_cur[tag] = t_new
                    for _, tag in specs:
                        pst = psp.tile([N, N], F32R, tag="ps", bufs=8,
                                       name=f"ptt_{tag}")
                        nc.tensor.transpose(pst[:], t_cur[tag][:], ident_r32[:])
                        tt_new = wp.tile([N, N], F32R, tag=f"TT_{tag}",
                                         bufs=2, name=f"TT_{tag}")
                        nc.scalar.copy(tt_new[:], pst[:])
                        tt_cur[tag] = tt_new
                    for _ in range(EXPM_SCAL):
                        for _, tag in specs:
                            psa = psp.tile([N, N], F32, tag="ps", bufs=8)
                            psb = psp.tile([N, N], F32, tag="ps", bufs=8)
                            nc.tensor.matmul(
                                psa[:], tt_cur[tag][:], t_cur[tag][:],
                                start=True, stop=True,
                            )
                            nc.tensor.matmul(
                                psb[:], t_cur[tag][:], tt_cur[tag][:],
                                start=True, stop=True,
                            )
                            t_new = wp.tile([N, N], F32R, tag=f"T_{tag}",
                                            bufs=2, name=f"T_{tag}")
                            tt_new = wp.tile([N, N], F32R, tag=f"TT_{tag}",
                                             bufs=2, name=f"TT_{tag}")
                            nc.vector.tensor_copy(t_new[:], psa[:])
                            nc.scalar.copy(tt_new[:], psb[:])
                            t_cur[tag], tt_cur[tag] = t_new, tt_new
                    return tt_cur

                eres = expm_batch([
                    (zt["ZC_U"], "UCT"), (zt["ZC_V"], "VCT"),
                    (zt["ZB_U"], "UBT"), (zt["ZB_V"], "VBT"),
                    (zt["ZA_U"], "UAT"),
                ])
                uct, vct = eres["UCT"], eres["VCT"]
                ubt, vbt = eres["UBT"], eres["VBT"]
                uat = eres["UAT"]

                def absdiag_col(z_tile, tag):
                    tmp = wp.tile([N, N], F32, tag="us")
                    nc.vector.tensor_mul(tmp[:], z_tile[:], ident[:])
                    col = sp.tile([N, 1], F32, tag=tag, name=f"col_{tag}")
                    nc.vector.tensor_reduce(
                        col[:], tmp[:], AXIS.X, ALU.add, apply_absolute_value=True
                    )
                    return col

                dc_col = absdiag_col(zt["ZC_S"], "dc")
                db_col = absdiag_col(zt["ZB_S"], "db")
                ga_col = absdiag_col(zt["ZA_G"], "ga")

                # top-4: alpha = sqrt(sum_i (b_i c_i)^2), b/c sorted desc.
                bwork = sp.tile([N, 1], F32, tag="bwork")
                cwork = sp.tile([N, 1], F32, tag="cwork")
                nc.vector.tensor_copy(bwork[:], db_col[:])
                nc.vector.tensor_copy(cwork[:], dc_col[:])
                acc = sp.tile([N, 1], F32, tag="acc")
                nc.vector.memset(acc[:], 0.0)
                bmax = sp.tile([N, 1], F32, tag="bmax")
                cmax = sp.tile([N, 1], F32, tag="cmax")
                prod = sp.tile([N, 1], F32, tag="prod")
                gmask = sp.tile([N, 1], F32, tag="gmask")
                tdrop = sp.tile([N, 1], F32, tag="tdrop")
                for i in range(KTOP):
                    nc.gpsimd.partition_all_reduce(
                        bmax[:], bwork[:], N, bass_isa.ReduceOp.max
                    )
                    nc.gpsimd.partition_all_reduce(
                        cmax[:], cwork[:], N, bass_isa.ReduceOp.max
                    )
                    nc.vector.tensor_mul(prod[:], bmax[:], cmax[:])
                    nc.vector.tensor_mul(prod[:], prod[:], prod[:])
                    nc.vector.tensor_add(acc[:], acc[:], prod[:])
                    if i < KTOP - 1:
                        nc.vector.tensor_single_scalar(
                            gmask[:], bwork[:], bmax[:], ALU.is_ge
                        )
                        nc.vector.tensor_mul(tdrop[:], bwork[:], gmask[:])
                        nc.vector.tensor_sub(bwork[:], bwork[:], tdrop[:])
                        nc.vector.tensor_single_scalar(
                            gmask[:], cwork[:], cmax[:], ALU.is_ge
                        )
                        nc.vector.tensor_mul(tdrop[:], cwork[:], gmask[:])
                        nc.vector.tensor_sub(cwork[:], cwork[:], tdrop[:])
                alpha = sp.tile([N, 1], F32, tag="alpha")
                nc.scalar.activation(alpha[:], acc[:], ACTF.Sqrt)

                sa05 = sp.tile([N, 1], F32, tag="sa05")
                nc.vector.tensor_scalar(
                    sa05[:], ga_col[:], alpha[:], -0.5, op0=ALU.add, op1=ALU.mult
                )
                sb01 = sp.tile([N, 1], F32, tag="sb01")
                nc.vector.tensor_scalar_mul(sb01[:], db_col[:], STEP)

                # C^T = VC @ (SC @ UC^T)
                p1 = wp.tile([N, N], F32R, tag="us_r", name="p1")
                nc.vector.tensor_scalar_mul(p1[:], uct[:], dc_col[:])
                psa = psp.tile([N, N], F32, tag="ps", bufs=8)
                nc.tensor.matmul(psa[:], vct[:], p1[:], start=True, stop=True)
                nc.vector.tensor_copy(CTf32[:], psa[:])

                # G^T = (0.01 B)^T = VB @ (0.01 SB @ UB^T)
                p2 = wp.tile([N, N], F32R, tag="us_r", name="p2")
                nc.vector.tensor_scalar_mul(p2[:], ubt[:], sb01[:])
                psb = psp.tile([N, N], F32, tag="ps", bufs=8)
                nc.tensor.matmul(psb[:], vbt[:], p2[:], start=True, stop=True)
                nc.vector.tensor_copy(BpTm[:], psb[:])
                W1T = ep.tile([N, N], F32, tag="W1T")      # G^T fp32
                nc.scalar.copy(W1T[:], psb[:])
                # untransposed G = 0.01 B = UB @ (0.01 SB @ VB^T)
                p2b = wp.tile([N, N], F32R, tag="us_r", name="p2b")
                nc.vector.tensor_scalar_mul(p2b[:], vbt[:], sb01[:])
                psb2 = psp.tile([N, N], F32, tag="ps", bufs=8)
                nc.tensor.matmul(psb2[:], ubt[:], p2b[:], start=True, stop=True)
                bp_un = ep.tile([N, N], F32, tag="Bpun")
                nc.vector.tensor_copy(bp_un[:], psb2[:])

                # A: M = UA @ (sa05 * UA^T); YA part via masked transpose
                p3 = wp.tile([N, N], F32R, tag="us_r", name="p3")
                nc.vector.tensor_scalar_mul(p3[:], uat[:], sa05[:])
                psm = psp.tile([N, N], F32, tag="ps", bufs=8)
                nc.tensor.matmul(psm[:], uat[:], p3[:], start=True, stop=True)
                uy = wp.tile([N, N], F32, tag="us")
                nc.vector.tensor_mul(uy[:], zt["ZA_Y"][:], masku[:])
                pst2 = psp.tile([N, N], F32, tag="ps", bufs=8)
                nc.tensor.transpose(pst2[:], uy[:], ident[:])
                nc.vector.tensor_scalar_mul(uy[:], uy[:], 0.5 * STEP)
                q2 = wp.tile([N, N], F32, tag="T")
                nc.vector.scalar_tensor_tensor(
                    q2[:], pst2[:], 0.5 * STEP, uy[:], op0=ALU.mult, op1=ALU.subtract
                )
                # (0.01 A)^T fp32 + fp16; untransposed 0.01 A fp32
                A01Tf = ep.tile([N, N], F32, tag="A01Tf")
                nc.vector.scalar_tensor_tensor(
                    A01Tf[:], psm[:], STEP, q2[:], op0=ALU.mult, op1=ALU.add
                )
                nc.vector.tensor_copy(A01Tm[:], A01Tf[:])
                a01_un = ep.tile([N, N], F32, tag="A01un")
                nc.vector.scalar_tensor_tensor(
                    a01_un[:], psm[:], STEP, q2[:], op0=ALU.mult, op1=ALU.subtract
                )

                def emit_mm(x_tile, tagname):
                    ps = psp.tile([N, N], F32, tag="ps", bufs=8, name=f"ps_{tagname}")
                    nc.tensor.matmul(ps[:], a01_un[:], x_tile[:], start=True, stop=True)
                    return ps

                def emit_gmm(x_tile, tagname):
                    ps = psp.tile([N, N], F32, tag="ps", bufs=8, name=f"pg_{tagname}")
                    nc.tensor.matmul(ps[:], bp_un[:], x_tile[:], start=True, stop=True)
                    return ps

                def fin_at(ps, x_tile, out_tile):
                    """out = x + psum  (the At^T multiply-add tail)."""
                    nc.vector.scalar_tensor_tensor(
                        out_tile[:], ps[:], 1.0, x_tile[:], op0=ALU.mult, op1=ALU.add
                    )

                # Chains T_k = (At^T)^k C^T and U_k = (At^T)^k (0.01A)^T are
                # independent; emit the waves interleaved so neither chain
                # head-blocks the in-order PE queue on the other's DVE tail.
                T1 = ep.tile([N, N], F32, tag="T1")
                U1 = ep.tile([N, N], F32, tag="U1")
                W2T = ep.tile([N, N], F32, tag="W2T")
                ps_t1 = emit_mm(CTf32, "T1")
                ps_u1 = emit_mm(A01Tf, "U1")
                ps_w2 = emit_gmm(A01Tf, "W2")
                fin_at(ps_t1, CTf32, T1)
                fin_at(ps_u1, A01Tf, U1)
                nc.vector.tensor_copy(W2T[:], ps_w2[:])

                T2 = ep.tile([N, N], F32, tag="T2")
                U2 = ep.tile([N, N], F32, tag="U2")
                W3T = ep.tile([N, N], F32, tag="W3T")
                ps_t2 = emit_mm(T1, "T2")
                ps_u2 = emit_mm(U1, "U2")
                ps_w3 = emit_gmm(U1, "W3")
                fin_at(ps_t2, T1, T2)
                fin_at(ps_u2, U1, U2)
                nc.vector.tensor_copy(W3T[:], ps_w3[:])

                T3 = ep.tile([N, N], F32, tag="T3")
                U3 = ep.tile([N, N], F32, tag="U3")
                W4T = ep.tile([N, N], F32, tag="W4T")
                ps_t3 = emit_mm(T2, "T3")
                ps_u3 = emit_mm(U2, "U3")
                ps_w4 = emit_gmm(U2, "W4")
                fin_at(ps_t3, T2, T3)
                fin_at(ps_u3, U2, U3)
                nc.vector.tensor_copy(A4Tm[:], U3[:])
                nc.vector.tensor_copy(W4T[:], ps_w4[:])

                T4 = ep.tile([N, N], F32, tag="T4")
                ssum = wp.tile([N, N], F32, tag="us")
                nc.vector.tensor_add(ssum[:], CTf32[:], T1[:])
                nc.vector.tensor_add(ssum[:], ssum[:], T2[:])
                nc.vector.tensor_add(ssum[:], ssum[:], T3[:])
                ps_t4 = emit_mm(T3, "T4")
                ps_q4 = emit_gmm(ssum, "Q4")
                fin_at(ps_t4, T3, T4)
                nc.vector.tensor_copy(P4Tm[:], T4[:])
                nc.vector.tensor_copy(Q4Tm[:], ps_q4[:])

                # GA = sum_j W_j*(1+(4-j)/4); GB = -sum_j W_j*(4-j)/4
                # G4 (plain) = sum_j W_j
                acc1 = wp.tile([N, N], F32, tag="us")
                nc.vector.tensor_scalar_mul(acc1[:], W1T[:], 1.75)
                nc.vector.scalar_tensor_tensor(
                    acc1[:], W2T[:], 1.5, acc1[:], op0=ALU.mult, op1=ALU.add
                )
                nc.vector.scalar_tensor_tensor(
                    acc1[:], W3T[:], 1.25, acc1[:], op0=ALU.mult, op1=ALU.add
                )
                nc.vector.scalar_tensor_tensor(
                    acc1[:], W4T[:], 1.0, acc1[:], op0=ALU.mult, op1=ALU.add
                )
                nc.vector.tensor_copy(GATm[:], acc1[:])
                acc2 = wp.tile([N, N], F32, tag="T")
                nc.vector.tensor_scalar_mul(acc2[:], W1T[:], -0.75)
                nc.vector.scalar_tensor_tensor(
                    acc2[:], W2T[:], -0.5, acc2[:], op0=ALU.mult, op1=ALU.add
                )
                nc.vector.scalar_tensor_tensor(
                    acc2[:], W3T[:], -0.25, acc2[:], op0=ALU.mult, op1=ALU.add
                )
                nc.vector.tensor_copy(GBTm[:], acc2[:])
                acc3 = wp.tile([N, N], F32, tag="us", name="acc3")
                nc.vector.tensor_add(acc3[:], W1T[:], W2T[:])
                nc.vector.tensor_add(acc3[:], acc3[:], W3T[:])
                nc.vector.tensor_add(acc3[:], acc3[:], W4T[:])
                nc.vector.tensor_copy(G4Tm[:], acc3[:])

                # vectors: c = 0.01 bx; w1 = At c, w2 = At w1, w3 = At w2
                # r4 = C (c+w1+w2+w3) + by
                # cc = c + 0.01A (c+w1+w2)
                def atv(v_in, tagname):
                    ps = psp.tile([N, 1], F32, tag="ps", bufs=8, name=f"pv_{tagname}")
                    nc.tensor.matmul(ps[:], A01Tf[:], v_in[:], start=True, stop=True)
                    v_out = sp.tile([N, 1], F32, tag=tagname)
                    nc.vector.scalar_tensor_tensor(
                        v_out[:], ps[:], 1.0, v_in[:], op0=ALU.mult, op1=ALU.add
                    )
                    return v_out

                w1 = atv(bxp_c, "w1")
                w2 = atv(w1, "w2")
                w3 = atv(w2, "w3")
                vs = sp.tile([N, 1], F32, tag="vs")
                nc.vector.tensor_add(vs[:], bxp_c[:], w1[:])
                vs2 = sp.tile([N, 1], F32, tag="vs2")
                nc.vector.tensor_add(vs2[:], vs[:], w2[:])
                vs3 = sp.tile([N, 1], F32, tag="vs3")
                nc.vector.tensor_add(vs3[:], vs2[:], w3[:])
                psr4 = psp.tile([N, 1], F32, tag="ps", bufs=8, name="psr4")
                nc.tensor.matmul(psr4[:], CTf32[:], vs3[:], start=True, stop=True)
                nc.vector.scalar_tensor_tensor(
                    r4_c[:], psr4[:], 1.0, by_c[:], op0=ALU.mult, op1=ALU.add
                )
                pscc = psp.tile([N, 1], F32, tag="ps", bufs=8, name="pscc")
                nc.tensor.matmul(pscc[:], A01Tf[:], vs2[:], start=True, stop=True)
                nc.vector.scalar_tensor_tensor(
                    cc_c[:], pscc[:], 1.0, bxp_c[:], op0=ALU.mult, op1=ALU.add
                )
                cc2_c = constp.tile([N, 1], F32, tag="cc2c")
                nc.vector.tensor_scalar_mul(cc2_c[:], cc_c[:], 2.0)

            # ------- recurrence: lag-4, paired steps -------
            with (
                tc.tile_pool(name="xrb", bufs=2) as xrbp,
                tc.tile_pool(name="ub", bufs=2) as ubp,
                tc.tile_pool(name="xb", bufs=4) as xbp,
                tc.tile_pool(name="dx", bufs=3) as dxp,
                tc.tile_pool(name="psy", bufs=3, space="PSUM") as psyp,
                tc.tile_pool(name="psx", bufs=3, space="PSUM") as psxp,
            ):
                CH = tc_chunk
                nchunks = tmax // CH

                xrb = xrbp.tile([N, CH * BSH], mdt, tag="xrb")
                ub = ubp.tile([N, CH * BSH], mdt, tag="ub")
                xr_bufs = {0: xrb}
                ub_bufs = {0: ub}

                # ---- bootstrap steps 0..3 (exact per-step form) ----
                nc.vector.tensor_copy(xrb[:, 0:BSH], x0_c[:])
                xb_cur = x0_c
                for k in range(4):
                    psyb = psyp.tile([N, 2 * BSH], F32, tag="psy", name=f"psyb{k}")
                    nc.tensor.matmul(
                        psyb[:, 0:BSH], CTf32[:], xb_cur[:], start=True, stop=True
                    )
                    nc.scalar.activation(
                        ub[:, ds(k * BSH, BSH)], psyb[:, 0:BSH], ACTF.Tanh,
                        bias=by_c[:], scale=1.0,
                    )
                    if k < 3:
                        psxb = psxp.tile([N, 2 * BSH], F32, tag="psx",
                                         name=f"psxb{k}")
                        nc.tensor.matmul(
                            psxb[:, 0:BSH], A01Tm[:], xrb[:, ds(k * BSH, BSH)],
                            start=True, stop=False,
                        )
                        nc.tensor.matmul(
                            psxb[:, 0:BSH], BpTm[:], ub[:, ds(k * BSH, BSH)],
                            start=False, stop=True,
                        )
                        xb_new = xbp.tile([N, BSH], F32, tag="xb", name=f"xbb{k}")
                        nc.vector.scalar_tensor_tensor(
                            xb_new[:], psxb[:, 0:BSH], bxp_c[:], xb_cur[:],
                            op0=ALU.add, op1=ALU.add,
                        )
                        nc.vector.scalar_tensor_tensor(
                            xrb[:, ds((k + 1) * BSH, BSH)], psxb[:, 0:BSH],
                            bxp_c[:], xb_cur[:], op0=ALU.add, op1=ALU.add,
                        )
                        xb_cur = xb_new
                # xb_cur == x_3 (the odd-step fp32 carry)

                # ---- main pair loop: t = 4, 6, ..., tmax-2 ----
                for t in range(4, tmax, 2):
                    s = t % CH
                    if s == 0:
                        cidx = t // CH
                        xrb = xrbp.tile([N, CH * BSH], mdt, tag="xrb")
                        ub = ubp.tile([N, CH * BSH], mdt, tag="ub")
                        xr_bufs[cidx] = xrb
                        ub_bufs[cidx] = ub
                        xr_bufs.pop(cidx - 2, None)
                        ub_bufs.pop(cidx - 2, None)

                    def pslice(bufs, tt):
                        b = bufs[tt // CH]
                        return b[:, ds((tt % CH) * BSH, 2 * BSH)]

                    xr4 = pslice(xr_bufs, t - 4)
                    u4 = pslice(ub_bufs, t - 4)

                    # psx pair: columns [delta_t - cc | delta_{t+1} - cc]
                    psx = psxp.tile([N, 2 * BSH], F32, tag="psx")
                    if t >= 8:
                        u8 = pslice(ub_bufs, t - 8)
                        nc.tensor.matmul(psx[:], GATm[:], u4, start=True, stop=False)
                        nc.tensor.matmul(psx[:], GBTm[:], u8, start=False, stop=False)
                        nc.tensor.matmul(psx[:], A4Tm[:], xr4, start=False, stop=True)
                    else:
                        nc.tensor.matmul(psx[:], G4Tm[:], u4, start=True, stop=False)
                        nc.tensor.matmul(psx[:], A4Tm[:], xr4, start=False, stop=True)
                    # psy pair -> tanh pair
                    psy = psyp.tile([N, 2 * BSH], F32, tag="psy")
                    nc.tensor.matmul(psy[:], Q4Tm[:], u4, start=True, stop=False)
                    nc.tensor.matmul(psy[:], P4Tm[:], xr4, start=False, stop=True)

                    # ACT: delta_t evacuation (x-loop critical), then tanh pair
                    # (cc is folded into the DVE ops: Copy takes no AP bias)
                    dxe = dxp.tile([N, BSH], F32, tag="dxe")
                    nc.scalar.copy(dxe[:], psx[:, 0:BSH])
                    nc.scalar.activation(
                        ub[:, ds(s * BSH, 2 * BSH)], psy[:], ACTF.Tanh,
                        bias=r4_c[:], scale=1.0,
                    )

                    # DVE: x_t (fp16), pair-sum s2, x_{t+1} (fp16)
                    nc.vector.scalar_tensor_tensor(
                        xrb[:, ds(s * BSH, BSH)], dxe[:], cc_c[:], xb_cur[:],
                        op0=ALU.add, op1=ALU.add,
                    )
                    s2 = dxp.tile([N, BSH], F32, tag="s2")
                    nc.vector.scalar_tensor_tensor(
                        s2[:], psx[:, ds(BSH, BSH)], cc2_c[:], dxe[:],
                        op0=ALU.add, op1=ALU.add,
                    )
                    # fp32 pair carry + x_{t+1} fp16 write both on GPSIMD
                    # (SBUF-only inputs) -- frees DVE, which is otherwise
                    # saturated alongside PE. Carry FIRST: the next pair's
                    # DVE ops read it one pair later, while the fp16 state
                    # write has two pairs of slack.
                    xb_new = xbp.tile([N, BSH], F32, tag="xb")
                    nc.gpsimd.tensor_add(xb_new[:], s2[:], xb_cur[:])
                    nc.gpsimd.tensor_add(
                        xrb[:, ds((s + 1) * BSH, BSH)], s2[:], xb_cur[:]
                    )
                    xb_cur = xb_new

                    if s + 2 == CH:
                        c = t // CH
                        nc.sync.dma_start(
                            out=out[:, ds(c * CH * BSH, CH * BSH)],
                            in_=xrb[:, 0:CH * BSH],
                        )

    nc.compile()
    return nc


_CACHED = {}


def _get_program(tmax=TMAX, tc_chunk=64, mdt=FP16):
    key = (tmax, tc_chunk, str(mdt))
    if key not in _CACHED:
        _CACHED[key] = build_program(tmax, tc_chunk, mdt)
    return _CACHED[key]


def make_in_maps(inputs, tmax=TMAX):
    X0 = np.ascontiguousarray(np.asarray(inputs["X0"], dtype=np.float32))
    base = {
        name: np.ascontiguousarray(np.asarray(inputs[name], dtype=np.float32))
        for name in PARAM_NAMES
    }
    base["bx"] = np.ascontiguousarray(
        np.asarray(inputs["bx"], dtype=np.float32).reshape(N, 1)
    )
    base["by"] = np.ascontiguousarray(
        np.asarray(inputs["by"], dtype=np.float32).reshape(N, 1)
    )
    in_maps = []
    for c in range(NCORES):
        m = dict(base)
        m["x0"] = np.ascontiguousarray(X0[c * BSH:(c + 1) * BSH].T)
        in_maps.append(m)
    return in_maps


def run_spmd(inputs, tmax=TMAX, tc_chunk=64, trace=False, tmpdir=None, mdt=FP16):
    nc = _get_program(tmax, tc_chunk, mdt)
    in_maps = make_in_maps(inputs, tmax)
    res = run_bass_kernel_spmd(
        nc, in_maps, list(range(NCORES)), trace=trace, tmpdir=tmpdir
    )
    X0 = np.asarray(inputs["X0"], dtype=np.float32)
    outs = []
    for c in range(NCORES):
        o = np.asarray(res.results[c]["out"])        # [N, tmax*BSH] fp16
        o = o.reshape(N, tmax, BSH).transpose(2, 1, 0).astype(np.float32)
        outs.append(o)                               # (BSH, tmax, N)
    full = np.concatenate(outs, axis=0)              # (BS, tmax, N)
    full[:, 0, :] = X0                               # exact t=0 plane
    return full, res


def kernel(**inputs):
    full, _ = run_spmd(inputs)
    return full


# revision 19
# speedup vs baseline: 1.1081x; 1.1081x over previous
"""LurieNet-k Trainium2 kernel (lag-4 paired recurrence, fp16 operands).

Computes, from the raw parametrization tensors, the matrices
  C = UC @ SC @ VC^T,  B = UB @ SB @ VB^T,
  A = 0.5*UA @ SA @ UA^T + 0.5*YA  (SA = -(alpha_upp*I + GA))
entirely on device (matrix exponentials of skew matrices via
scaling-and-squaring Taylor), then runs the 511-step recurrence
  u_t = tanh(C x_t + by);  x_{t+1} = x_t + 0.01*(A x_t + B u_t + bx)
on a (128, 64) state shard per NeuronCore (batch data-parallel over
the 8 cores).

Structure: the naive step is a serial tanh->matmul->tanh round trip
(~860ns on TRN2: ACT access latency + 2 sem hops + PE). Because the
tanh self-coupling Q = 0.01*C*B has tiny norm (~3e-4), the recurrence
is re-expanded to an (almost) exact LAG-4 form: every quantity at
step t is computed from state/tanh values at steps t-4/t-3 (and t-8
/t-7 for a first-order staleness extrapolation of the x-chain's u
terms, which kills the dominant scheme error: measured 1.2e-2 plain
-> 3.2e-4 extrapolated, fp16). All matmul inputs are then >= 2
pair-iterations old, so nothing serializes, and steps are processed
in PAIRS:
  - one 128-wide matmul per weight per pair (halves LDWEIGHTS, the
    PE throughput limit)
  - one 128-wide tanh per pair (halves ACT's per-instruction access
    latency tax)
  - PSUM evacuation split across ACT (Copy w/ bias for delta_t),
    DVE (pair-sum + the two fp16 state writes), and GPSIMD (fp32
    pair carry, SBUF-only since GPSIMD cannot read PSUM).
Weights/states/tanh values are fp16 (same PE speed as bf16, 8x finer
rounding); the fp32 carry keeps the state exact (all partial sums
fp32). Output is written fp16 time-major [n, t, b] straight from the
state buffer (no on-device transpose) and transposed to (b, t, n)
fp32 on the host during unsharding; the t=0 plane is restored
exactly from X0. Measured end-to-end rel err ~4e-4 (budget 2e-2).
"""

import sys

for _p in ("/opt/trn_rl_repo",):
    if _p not in sys.path:
        sys.path.insert(0, _p)

import numpy as np

import concourse.bass as bass
import concourse.mybir as mybir
import concourse.tile as tile
from concourse import bacc
from concourse import bass_isa
from concourse.bass import ds
from concourse.bass_utils import run_bass_kernel_spmd
from concourse.masks import make_identity, make_upper_triangular

F32 = mybir.dt.float32
F32R = mybir.dt.float32r
FP16 = mybir.dt.float16
ALU = mybir.AluOpType
ACTF = mybir.ActivationFunctionType
AXIS = mybir.AxisListType

N = 128          # state dim
TMAX = 512       # time steps (including t=0)
BS = 512         # global batch
NCORES = 8
BSH = BS // NCORES   # 64 batch columns per core
STEP = 0.01
KTOP = 4

EXPM_SCAL = 3    # expm scaling: X = S / 2**EXPM_SCAL, then 3 squarings
EXPM_TERMS = 4   # Taylor terms in the Horner evaluation

PARAM_NAMES = [
    "ZA_Y", "ZA_U", "ZA_G", "ZB_U", "ZB_V", "ZB_S", "ZC_U", "ZC_V", "ZC_S",
]


def build_program(tmax=TMAX, tc_chunk=64, mdt=FP16):
    """Build the single-NeuronCore Bass program (run SPMD on all 8 cores)."""
    assert tmax % tc_chunk == 0 and tc_chunk % 2 == 0
    nc = bacc.Bacc(
        "TRN2",
        target_bir_lowering=False,
        debug=False,
        enable_asserts=False,
        num_devices=NCORES,
    )

    x0 = nc.dram_tensor("x0", [N, BSH], F32, kind="ExternalInput")
    zs = {
        name: nc.dram_tensor(name, [N, N], F32, kind="ExternalInput")
        for name in PARAM_NAMES
    }
    bx_d = nc.dram_tensor("bx", [N, 1], F32, kind="ExternalInput")
    by_d = nc.dram_tensor("by", [N, 1], F32, kind="ExternalInput")
    # time-major fp16 output: out[n, t*BSH + b]; host transposes to (b,t,n)
    out = nc.dram_tensor("out", [N, tmax * BSH], FP16, kind="ExternalOutput")

    with tile.TileContext(nc) as tc:
        with tc.tile_pool(name="const", bufs=1) as constp:
            ident = constp.tile([N, N], F32, tag="ident")
            make_identity(nc, ident[:])
            masku = constp.tile([N, N], F32, tag="masku")
            make_upper_triangular(nc, masku[:], val=1.0, diag=False)
            ident_r32 = constp.tile([N, N], F32R, tag="ident_r32")
            nc.vector.tensor_copy(ident_r32[:], ident[:])

            by_c = constp.tile([N, 1], F32, tag="by")
            nc.sync.dma_start(out=by_c[:], in_=by_d[:])
            bx_c = constp.tile([N, 1], F32, tag="bxraw")
            nc.sync.dma_start(out=bx_c[:], in_=bx_d[:])
            bxp_c = constp.tile([N, 1], F32, tag="bxp")
            nc.vector.tensor_scalar_mul(bxp_c[:], bx_c[:], STEP)
            x0_c = constp.tile([N, BSH], F32, tag="x0c")
            nc.sync.dma_start(out=x0_c[:], in_=x0[:])

            # ------- runtime weights (transposed, fp16) -------
            P4Tm = constp.tile([N, N], mdt, tag="P4Tm")    # (C At^4)^T
            Q4Tm = constp.tile([N, N], mdt, tag="Q4Tm")    # (C S3 G)^T
            A4Tm = constp.tile([N, N], mdt, tag="A4Tm")    # (0.01A At^3)^T
            GATm = constp.tile([N, N], mdt, tag="GATm")    # extrap u_{t-4} w
            GBTm = constp.tile([N, N], mdt, tag="GBTm")    # extrap u_{t-8} w
            G4Tm = constp.tile([N, N], mdt, tag="G4Tm")    # plain (boot pairs)
            A01Tm = constp.tile([N, N], mdt, tag="A01Tm")  # (0.01 A)^T (boot)
            BpTm = constp.tile([N, N], mdt, tag="BpTm")    # (0.01 B)^T (boot)
            CTf32 = constp.tile([N, N], F32, tag="CTf32")  # C^T fp32 (boot)
            r4_c = constp.tile([N, 1], F32, tag="r4c")     # C S3 c + by
            cc_c = constp.tile([N, 1], F32, tag="ccc")     # delta const
            # quad (lag-8) weights
            P8Tm = constp.tile([N, N], mdt, tag="P8Tm")    # (C At^8)^T
            Q8Tm = constp.tile([N, N], mdt, tag="Q8Tm")    # (C S7 G)^T
            A8Tm = constp.tile([N, N], mdt, tag="A8Tm")    # (0.01A At^7)^T
            GA8Tm = constp.tile([N, N], mdt, tag="GA8Tm")  # extrap u_{t-8} w
            GB8Tm = constp.tile([N, N], mdt, tag="GB8Tm")  # extrap u_{t-16} w
            r8_c = constp.tile([N, 1], F32, tag="r8c")
            ccrow = constp.tile([1, N], F32R, tag="ccrow")   # cc8 as a row
            ones4b = constp.tile([1, 4 * BSH], F32R, tag="ones4b")
            ones4f = constp.tile([1, 4 * BSH], F32, tag="ones4f")
            nc.vector.memset(ones4f[:], 1.0)
            nc.vector.tensor_copy(ones4b[:], ones4f[:])

            with (
                tc.tile_pool(name="zbuf", bufs=1) as zp,
                tc.tile_pool(name="work", bufs=2) as wp,
                tc.tile_pool(name="eres", bufs=1) as ep,
                tc.tile_pool(name="small", bufs=1) as sp,
                tc.tile_pool(name="pss", bufs=4, space="PSUM") as psp,
            ):
                zt = {}
                for name in PARAM_NAMES:
                    zt[name] = zp.tile([N, N], F32, tag=name, name=f"z_{name}")
                    nc.sync.dma_start(out=zt[name][:], in_=zs[name][:])

                def expm_batch(specs):
                    """Interleaved expm(skew(Z))^T for all matrices at once.

                    Maintains the (T, T^T) pair through Horner + squaring so
                    no PE transposes are needed: with negX = X^T = -X,
                      X @ T     = matmul(lhsT=negX, rhs=T)
                      T^T @ X^T = matmul(lhsT=T,    rhs=negX)
                    """
                    scal = 1.0 / (2.0 ** EXPM_SCAL)
                    negx = {}
                    t_cur = {}
                    tt_cur = {}
                    for z_tile, tag in specs:
                        us = wp.tile([N, N], F32R, tag="us_r", name=f"us_{tag}")
                        nc.vector.scalar_tensor_tensor(
                            us[:], z_tile[:], scal, masku[:],
                            op0=ALU.mult, op1=ALU.mult,
                        )
                        pst = psp.tile([N, N], F32R, tag="ps", bufs=7,
                                       name=f"pst_{tag}")
                        nc.tensor.transpose(pst[:], us[:], ident_r32[:])
                        nx = wp.tile([N, N], F32R, tag=f"negx_{tag}", bufs=1,
                                     name=f"negx_{tag}")
                        nc.vector.scalar_tensor_tensor(
                            nx[:], pst[:], 1.0, us[:],
                            op0=ALU.mult, op1=ALU.subtract,
                        )
                        negx[tag] = nx
                        t_cur[tag] = ident_r32
                        tt_cur[tag] = ident_r32
                    for j in range(EXPM_TERMS, 0, -1):
                        for _, tag in specs:
                            psa = psp.tile([N, N], F32, tag="ps", bufs=7)
                            nc.tensor.matmul(
                                psa[:], negx[tag][:], t_cur[tag][:],
                                start=True, stop=True,
                            )
                            t_new = wp.tile([N, N], F32R, tag=f"T_{tag}",
                                            bufs=2, name=f"T_{tag}")
                            nc.vector.scalar_tensor_tensor(
                                t_new[:], psa[:], 1.0 / j, ident_r32[:],
                                op0=ALU.mult, op1=ALU.add,
                            )
                            t_cur[tag] = t_new
                    for _, tag in specs:
                        pst = psp.tile([N, N], F32R, tag="ps", bufs=7,
                                       name=f"ptt_{tag}")
                        nc.tensor.transpose(pst[:], t_cur[tag][:], ident_r32[:])
                        tt_new = wp.tile([N, N], F32R, tag=f"TT_{tag}",
                                         bufs=2, name=f"TT_{tag}")
                        nc.scalar.copy(tt_new[:], pst[:])
                        tt_cur[tag] = tt_new
                    for _ in range(EXPM_SCAL):
                        for _, tag in specs:
                            psa = psp.tile([N, N], F32, tag="ps", bufs=7)
                            psb = psp.tile([N, N], F32, tag="ps", bufs=7)
                            nc.tensor.matmul(
                                psa[:], tt_cur[tag][:], t_cur[tag][:],
                                start=True, stop=True,
                            )
                            nc.tensor.matmul(
                                psb[:], t_cur[tag][:], tt_cur[tag][:],
                                start=True, stop=True,
                            )
                            t_new = wp.tile([N, N], F32R, tag=f"T_{tag}",
                                            bufs=2, name=f"T_{tag}")
                            tt_new = wp.tile([N, N], F32R, tag=f"TT_{tag}",
                                             bufs=2, name=f"TT_{tag}")
                            nc.vector.tensor_copy(t_new[:], psa[:])
                            nc.scalar.copy(tt_new[:], psb[:])
                            t_cur[tag], tt_cur[tag] = t_new, tt_new
                    return tt_cur

                eres = expm_batch([
                    (zt["ZC_U"], "UCT"), (zt["ZC_V"], "VCT"),
                    (zt["ZB_U"], "UBT"), (zt["ZB_V"], "VBT"),
                    (zt["ZA_U"], "UAT"),
                ])
                uct, vct = eres["UCT"], eres["VCT"]
                ubt, vbt = eres["UBT"], eres["VBT"]
                uat = eres["UAT"]

                def absdiag_col(z_tile, tag):
                    tmp = wp.tile([N, N], F32, tag="us")
                    nc.vector.tensor_mul(tmp[:], z_tile[:], ident[:])
                    col = sp.tile([N, 1], F32, tag=tag, name=f"col_{tag}")
                    nc.vector.tensor_reduce(
                        col[:], tmp[:], AXIS.X, ALU.add, apply_absolute_value=True
                    )
                    return col

                dc_col = absdiag_col(zt["ZC_S"], "dc")
                db_col = absdiag_col(zt["ZB_S"], "db")
                ga_col = absdiag_col(zt["ZA_G"], "ga")

                # top-4: alpha = sqrt(sum_i (b_i c_i)^2), b/c sorted desc.
                bwork = sp.tile([N, 1], F32, tag="bwork")
                cwork = sp.tile([N, 1], F32, tag="cwork")
                nc.vector.tensor_copy(bwork[:], db_col[:])
                nc.vector.tensor_copy(cwork[:], dc_col[:])
                acc = sp.tile([N, 1], F32, tag="acc")
                nc.vector.memset(acc[:], 0.0)
                bmax = sp.tile([N, 1], F32, tag="bmax")
                cmax = sp.tile([N, 1], F32, tag="cmax")
                prod = sp.tile([N, 1], F32, tag="prod")
                gmask = sp.tile([N, 1], F32, tag="gmask")
                tdrop = sp.tile([N, 1], F32, tag="tdrop")
                for i in range(KTOP):
                    nc.gpsimd.partition_all_reduce(
                        bmax[:], bwork[:], N, bass_isa.ReduceOp.max
                    )
                    nc.gpsimd.partition_all_reduce(
                        cmax[:], cwork[:], N, bass_isa.ReduceOp.max
                    )
                    nc.vector.tensor_mul(prod[:], bmax[:], cmax[:])
                    nc.vector.tensor_mul(prod[:], prod[:], prod[:])
                    nc.vector.tensor_add(acc[:], acc[:], prod[:])
                    if i < KTOP - 1:
                        nc.vector.tensor_single_scalar(
                            gmask[:], bwork[:], bmax[:], ALU.is_ge
                        )
                        nc.vector.tensor_mul(tdrop[:], bwork[:], gmask[:])
                        nc.vector.tensor_sub(bwork[:], bwork[:], tdrop[:])
                        nc.vector.tensor_single_scalar(
                            gmask[:], cwork[:], cmax[:], ALU.is_ge
                        )
                        nc.vector.tensor_mul(tdrop[:], cwork[:], gmask[:])
                        nc.vector.tensor_sub(cwork[:], cwork[:], tdrop[:])
                alpha = sp.tile([N, 1], F32, tag="alpha")
                nc.scalar.activation(alpha[:], acc[:], ACTF.Sqrt)

                sa05 = sp.tile([N, 1], F32, tag="sa05")
                nc.vector.tensor_scalar(
                    sa05[:], ga_col[:], alpha[:], -0.5, op0=ALU.add, op1=ALU.mult
                )
                sb01 = sp.tile([N, 1], F32, tag="sb01")
                nc.vector.tensor_scalar_mul(sb01[:], db_col[:], STEP)

                # C^T = VC @ (SC @ UC^T)
                p1 = wp.tile([N, N], F32R, tag="us_r", name="p1")
                nc.vector.tensor_scalar_mul(p1[:], uct[:], dc_col[:])
                psa = psp.tile([N, N], F32, tag="ps", bufs=7)
                nc.tensor.matmul(psa[:], vct[:], p1[:], start=True, stop=True)
                nc.vector.tensor_copy(CTf32[:], psa[:])

                # G^T = (0.01 B)^T = VB @ (0.01 SB @ UB^T)
                p2 = wp.tile([N, N], F32R, tag="us_r", name="p2")
                nc.vector.tensor_scalar_mul(p2[:], ubt[:], sb01[:])
                psb = psp.tile([N, N], F32, tag="ps", bufs=7)
                nc.tensor.matmul(psb[:], vbt[:], p2[:], start=True, stop=True)
                nc.vector.tensor_copy(BpTm[:], psb[:])
                W1T = ep.tile([N, N], F32, tag="W1T")      # G^T fp32
                nc.scalar.copy(W1T[:], psb[:])
                # untransposed G = 0.01 B = UB @ (0.01 SB @ VB^T)
                p2b = wp.tile([N, N], F32R, tag="us_r", name="p2b")
                nc.vector.tensor_scalar_mul(p2b[:], vbt[:], sb01[:])
                psb2 = psp.tile([N, N], F32, tag="ps", bufs=7)
                nc.tensor.matmul(psb2[:], ubt[:], p2b[:], start=True, stop=True)
                bp_un = ep.tile([N, N], F32, tag="Bpun")
                nc.vector.tensor_copy(bp_un[:], psb2[:])

                # A: M = UA @ (sa05 * UA^T); YA part via masked transpose
                p3 = wp.tile([N, N], F32R, tag="us_r", name="p3")
                nc.vector.tensor_scalar_mul(p3[:], uat[:], sa05[:])
                psm = psp.tile([N, N], F32, tag="ps", bufs=7)
                nc.tensor.matmul(psm[:], uat[:], p3[:], start=True, stop=True)
                uy = wp.tile([N, N], F32, tag="us")
                nc.vector.tensor_mul(uy[:], zt["ZA_Y"][:], masku[:])
                pst2 = psp.tile([N, N], F32, tag="ps", bufs=7)
                nc.tensor.transpose(pst2[:], uy[:], ident[:])
                nc.vector.tensor_scalar_mul(uy[:], uy[:], 0.5 * STEP)
                q2 = wp.tile([N, N], F32, tag="T")
                nc.vector.scalar_tensor_tensor(
                    q2[:], pst2[:], 0.5 * STEP, uy[:], op0=ALU.mult, op1=ALU.subtract
                )
                # (0.01 A)^T fp32 + fp16; untransposed 0.01 A fp32
                A01Tf = ep.tile([N, N], F32, tag="A01Tf")
                nc.vector.scalar_tensor_tensor(
                    A01Tf[:], psm[:], STEP, q2[:], op0=ALU.mult, op1=ALU.add
                )
                nc.vector.tensor_copy(A01Tm[:], A01Tf[:])
                a01_un = ep.tile([N, N], F32, tag="A01un")
                nc.vector.scalar_tensor_tensor(
                    a01_un[:], psm[:], STEP, q2[:], op0=ALU.mult, op1=ALU.subtract
                )

                def emit_mm(x_tile, tagname):
                    ps = psp.tile([N, N], F32, tag="ps", bufs=7, name=f"ps_{tagname}")
                    nc.tensor.matmul(ps[:], a01_un[:], x_tile[:], start=True, stop=True)
                    return ps

                def emit_gmm(x_tile, tagname):
                    ps = psp.tile([N, N], F32, tag="ps", bufs=7, name=f"pg_{tagname}")
                    nc.tensor.matmul(ps[:], bp_un[:], x_tile[:], start=True, stop=True)
                    return ps

                def fin_at(ps, x_tile, out_tile):
                    """out = x + psum  (the At^T multiply-add tail)."""
                    nc.vector.scalar_tensor_tensor(
                        out_tile[:], ps[:], 1.0, x_tile[:], op0=ALU.mult, op1=ALU.add
                    )

                # Chains T_k = (At^T)^k C^T and U_k = (At^T)^k (0.01A)^T are
                # independent; emit the waves interleaved so neither chain
                # head-blocks the in-order PE queue on the other's DVE tail.
                T1 = ep.tile([N, N], F32, tag="T1")
                U1 = ep.tile([N, N], F32, tag="U1")
                W2T = ep.tile([N, N], F32, tag="W2T")
                ps_t1 = emit_mm(CTf32, "T1")
                ps_u1 = emit_mm(A01Tf, "U1")
                ps_w2 = emit_gmm(A01Tf, "W2")
                fin_at(ps_t1, CTf32, T1)
                fin_at(ps_u1, A01Tf, U1)
                nc.vector.tensor_copy(W2T[:], ps_w2[:])

                T2 = ep.tile([N, N], F32, tag="T2")
                U2 = ep.tile([N, N], F32, tag="U2")
                W3T = ep.tile([N, N], F32, tag="W3T")
                ps_t2 = emit_mm(T1, "T2")
                ps_u2 = emit_mm(U1, "U2")
                ps_w3 = emit_gmm(U1, "W3")
                fin_at(ps_t2, T1, T2)
                fin_at(ps_u2, U1, U2)
                nc.vector.tensor_copy(W3T[:], ps_w3[:])

                T3 = ep.tile([N, N], F32, tag="T3")
                U3 = ep.tile([N, N], F32, tag="U3")
                W4T = ep.tile([N, N], F32, tag="W4T")
                ps_t3 = emit_mm(T2, "T3")
                ps_u3 = emit_mm(U2, "U3")
                ps_w4 = emit_gmm(U2, "W4")
                fin_at(ps_t3, T2, T3)
                fin_at(ps_u3, U2, U3)
                nc.vector.tensor_copy(A4Tm[:], U3[:])
                nc.vector.tensor_copy(W4T[:], ps_w4[:])

                T4 = ep.tile([N, N], F32, tag="T4")
                U4 = ep.tile([N, N], F32, tag="U4")
                W5T = ep.tile([N, N], F32, tag="W5T")
                ssum = wp.tile([N, N], F32, tag="us")
                nc.vector.tensor_add(ssum[:], CTf32[:], T1[:])
                nc.vector.tensor_add(ssum[:], ssum[:], T2[:])
                nc.vector.tensor_add(ssum[:], ssum[:], T3[:])
                ps_t4 = emit_mm(T3, "T4")
                ps_q4 = emit_gmm(ssum, "Q4")
                ps_u4 = emit_mm(U3, "U4")
                ps_w5 = emit_gmm(U3, "W5")
                fin_at(ps_t4, T3, T4)
                nc.vector.tensor_copy(P4Tm[:], T4[:])
                nc.vector.tensor_copy(Q4Tm[:], ps_q4[:])
                fin_at(ps_u4, U3, U4)
                nc.vector.tensor_copy(W5T[:], ps_w5[:])

                # extend the chains to At^8 for the quad (lag-8) regime
                T5 = ep.tile([N, N], F32, tag="T5")
                U5 = ep.tile([N, N], F32, tag="U5")
                W6T = ep.tile([N, N], F32, tag="W6T")
                ps_t5 = emit_mm(T4, "T5")
                ps_u5 = emit_mm(U4, "U5")
                ps_w6 = emit_gmm(U4, "W6")
                fin_at(ps_t5, T4, T5)
                fin_at(ps_u5, U4, U5)
                nc.vector.tensor_copy(W6T[:], ps_w6[:])

                T6 = ep.tile([N, N], F32, tag="T6")
                U6 = ep.tile([N, N], F32, tag="U6")
                W7T = ep.tile([N, N], F32, tag="W7T")
                ps_t6 = emit_mm(T5, "T6")
                ps_u6 = emit_mm(U5, "U6")
                ps_w7 = emit_gmm(U5, "W7")
                fin_at(ps_t6, T5, T6)
                fin_at(ps_u6, U5, U6)
                nc.vector.tensor_copy(W7T[:], ps_w7[:])

                T7 = ep.tile([N, N], F32, tag="T7")
                U7 = ep.tile([N, N], F32, tag="U7")
                W8T = ep.tile([N, N], F32, tag="W8T")
                ps_t7 = emit_mm(T6, "T7")
                ps_u7 = emit_mm(U6, "U7")
                ps_w8 = emit_gmm(U6, "W8")
                fin_at(ps_t7, T6, T7)
                fin_at(ps_u7, U6, U7)
                nc.vector.tensor_copy(A8Tm[:], U7[:])
                nc.vector.tensor_copy(W8T[:], ps_w8[:])

                T8 = ep.tile([N, N], F32, tag="T8")
                nc.vector.tensor_add(ssum[:], ssum[:], T4[:])
                nc.vector.tensor_add(ssum[:], ssum[:], T5[:])
                nc.vector.tensor_add(ssum[:], ssum[:], T6[:])
                nc.vector.tensor_add(ssum[:], ssum[:], T7[:])
                ps_t8 = emit_mm(T7, "T8")
                ps_q8 = emit_gmm(ssum, "Q8")
                fin_at(ps_t8, T7, T8)
                nc.vector.tensor_copy(P8Tm[:], T8[:])
                nc.vector.tensor_copy(Q8Tm[:], ps_q8[:])

                # GA = sum_j W_j*(1+(4-j)/4); GB = -sum_j W_j*(4-j)/4
                # G4 (plain) = sum_j W_j
                acc1 = wp.tile([N, N], F32, tag="us")
                nc.vector.tensor_scalar_mul(acc1[:], W1T[:], 1.75)
                nc.vector.scalar_tensor_tensor(
                    acc1[:], W2T[:], 1.5, acc1[:], op0=ALU.mult, op1=ALU.add
                )
                nc.vector.scalar_tensor_tensor(
                    acc1[:], W3T[:], 1.25, acc1[:], op0=ALU.mult, op1=ALU.add
                )
                nc.vector.scalar_tensor_tensor(
                    acc1[:], W4T[:], 1.0, acc1[:], op0=ALU.mult, op1=ALU.add
                )
                nc.vector.tensor_copy(GATm[:], acc1[:])
                acc2 = wp.tile([N, N], F32, tag="T")
                nc.vector.tensor_scalar_mul(acc2[:], W1T[:], -0.75)
                nc.vector.scalar_tensor_tensor(
                    acc2[:], W2T[:], -0.5, acc2[:], op0=ALU.mult, op1=ALU.add
                )
                nc.vector.scalar_tensor_tensor(
                    acc2[:], W3T[:], -0.25, acc2[:], op0=ALU.mult, op1=ALU.add
                )
                nc.vector.tensor_copy(GBTm[:], acc2[:])
                acc3 = wp.tile([N, N], F32, tag="us", name="acc3")
                nc.vector.tensor_add(acc3[:], W1T[:], W2T[:])
                nc.vector.tensor_add(acc3[:], acc3[:], W3T[:])
                nc.vector.tensor_add(acc3[:], acc3[:], W4T[:])
                nc.vector.tensor_copy(G4Tm[:], acc3[:])

                # GA8 = sum_j W_j*(1+(8-j)/8); GB8 = -sum_j W_j*(8-j)/8
                WTs = [W1T, W2T, W3T, W4T, W5T, W6T, W7T, W8T]
                acc8a = wp.tile([N, N], F32, tag="us", name="acc8a")
                nc.vector.tensor_scalar_mul(acc8a[:], W1T[:], 1.875)
                for j in range(2, 9):
                    nc.vector.scalar_tensor_tensor(
                        acc8a[:], WTs[j - 1][:], 1.0 + (8.0 - j) / 8.0, acc8a[:],
                        op0=ALU.mult, op1=ALU.add,
                    )
                nc.vector.tensor_copy(GA8Tm[:], acc8a[:])
                acc8b = wp.tile([N, N], F32, tag="T", name="acc8b")
                nc.vector.tensor_scalar_mul(acc8b[:], W1T[:], -0.875)
                for j in range(2, 8):
                    nc.vector.scalar_tensor_tensor(
                        acc8b[:], WTs[j - 1][:], -(8.0 - j) / 8.0, acc8b[:],
                        op0=ALU.mult, op1=ALU.add,
                    )
                nc.vector.tensor_copy(GB8Tm[:], acc8b[:])

                # vectors: c = 0.01 bx; w1 = At c, w2 = At w1, w3 = At w2
                # r4 = C (c+w1+w2+w3) + by
                # cc = c + 0.01A (c+w1+w2)
                def atv(v_in, tagname):
                    ps = psp.tile([N, 1], F32, tag="ps", bufs=7, name=f"pv_{tagname}")
                    nc.tensor.matmul(ps[:], A01Tf[:], v_in[:], start=True, stop=True)
                    v_out = sp.tile([N, 1], F32, tag=tagname)
                    nc.vector.scalar_tensor_tensor(
                        v_out[:], ps[:], 1.0, v_in[:], op0=ALU.mult, op1=ALU.add
                    )
                    return v_out

                w1 = atv(bxp_c, "w1")
                w2 = atv(w1, "w2")
                w3 = atv(w2, "w3")
                w4 = atv(w3, "w4")
                w5 = atv(w4, "w5")
                w6 = atv(w5, "w6")
                w7 = atv(w6, "w7")
                vs = sp.tile([N, 1], F32, tag="vs")
                nc.vector.tensor_add(vs[:], bxp_c[:], w1[:])
                vs2 = sp.tile([N, 1], F32, tag="vs2")
                nc.vector.tensor_add(vs2[:], vs[:], w2[:])
                vs3 = sp.tile([N, 1], F32, tag="vs3")
                nc.vector.tensor_add(vs3[:], vs2[:], w3[:])
                psr4 = psp.tile([N, 1], F32, tag="ps", bufs=7, name="psr4")
                nc.tensor.matmul(psr4[:], CTf32[:], vs3[:], start=True, stop=True)
                nc.vector.scalar_tensor_tensor(
                    r4_c[:], psr4[:], 1.0, by_c[:], op0=ALU.mult, op1=ALU.add
                )
                pscc = psp.tile([N, 1], F32, tag="ps", bufs=7, name="pscc")
                nc.tensor.matmul(pscc[:], A01Tf[:], vs2[:], start=True, stop=True)
                nc.vector.scalar_tensor_tensor(
                    cc_c[:], pscc[:], 1.0, bxp_c[:], op0=ALU.mult, op1=ALU.add
                )
                cc2_c = constp.tile([N, 1], F32, tag="cc2c")
                nc.vector.tensor_scalar_mul(cc2_c[:], cc_c[:], 2.0)

                # lag-8 constants: r8 = C (sum_{k=0..7} At^k) c + by,
                # cc8 = c + 0.01A (sum_{k=0..6} At^k) c, exposed as a row
                # for the rank-1 (cc8 x ones) matmul into the quad psx.
                vs6 = sp.tile([N, 1], F32, tag="vs6")
                nc.vector.tensor_add(vs6[:], vs3[:], w4[:])
                nc.vector.tensor_add(vs6[:], vs6[:], w5[:])
                nc.vector.tensor_add(vs6[:], vs6[:], w6[:])
                vs7 = sp.tile([N, 1], F32, tag="vs7")
                nc.vector.tensor_add(vs7[:], vs6[:], w7[:])
                psr8 = psp.tile([N, 1], F32, tag="ps", bufs=7, name="psr8")
                nc.tensor.matmul(psr8[:], CTf32[:], vs7[:], start=True, stop=True)
                nc.vector.scalar_tensor_tensor(
                    r8_c[:], psr8[:], 1.0, by_c[:], op0=ALU.mult, op1=ALU.add
                )
                pscc8 = psp.tile([N, 1], F32, tag="ps", bufs=7, name="pscc8")
                nc.tensor.matmul(pscc8[:], A01Tf[:], vs6[:], start=True, stop=True)
                cc8_c = sp.tile([N, 1], F32, tag="cc8c")
                nc.vector.scalar_tensor_tensor(
                    cc8_c[:], pscc8[:], 1.0, bxp_c[:], op0=ALU.mult, op1=ALU.add
                )
                # transpose cc8 [N,1] -> [1,N] via PE, land as f32r row
                psccr = psp.tile([1, N], F32, tag="psrow", bufs=1, name="psccr")
                nc.tensor.transpose(psccr[:], cc8_c[:], ident[:])
                nc.vector.tensor_copy(ccrow[:], psccr[:])

            # ------- recurrence: lag-4, paired steps -------
            with (
                tc.tile_pool(name="xrb", bufs=2) as xrbp,
                tc.tile_pool(name="ub", bufs=2) as ubp,
                tc.tile_pool(name="xb", bufs=4) as xbp,
                tc.tile_pool(name="dx", bufs=3) as dxp,
                tc.tile_pool(name="psy", bufs=3, space="PSUM") as psyp,
                tc.tile_pool(name="psx", bufs=3, space="PSUM") as psxp,
            ):
                CH = tc_chunk
                nchunks = tmax // CH

                xrb = xrbp.tile([N, CH * BSH], mdt, tag="xrb")
                ub = ubp.tile([N, CH * BSH], mdt, tag="ub")
                xr_bufs = {0: xrb}
                ub_bufs = {0: ub}

                # ---- bootstrap steps 0..3 (exact per-step form) ----
                nc.vector.tensor_copy(xrb[:, 0:BSH], x0_c[:])
                xb_cur = x0_c
                for k in range(4):
                    psyb = psyp.tile([N, 4 * BSH], F32, tag="psy", name=f"psyb{k}")
                    nc.tensor.matmul(
                        psyb[:, 0:BSH], CTf32[:], xb_cur[:], start=True, stop=True
                    )
                    nc.scalar.activation(
                        ub[:, ds(k * BSH, BSH)], psyb[:, 0:BSH], ACTF.Tanh,
                        bias=by_c[:], scale=1.0,
                    )
                    if k < 3:
                        psxb = psxp.tile([N, 4 * BSH], F32, tag="psx",
                                         name=f"psxb{k}")
                        nc.tensor.matmul(
                            psxb[:, 0:BSH], A01Tm[:], xrb[:, ds(k * BSH, BSH)],
                            start=True, stop=False,
                        )
                        nc.tensor.matmul(
                            psxb[:, 0:BSH], BpTm[:], ub[:, ds(k * BSH, BSH)],
                            start=False, stop=True,
                        )
                        xb_new = xbp.tile([N, BSH], F32, tag="xb", name=f"xbb{k}")
                        nc.vector.scalar_tensor_tensor(
                            xb_new[:], psxb[:, 0:BSH], bxp_c[:], xb_cur[:],
                            op0=ALU.add, op1=ALU.add,
                        )
                        nc.vector.scalar_tensor_tensor(
                            xrb[:, ds((k + 1) * BSH, BSH)], psxb[:, 0:BSH],
                            bxp_c[:], xb_cur[:], op0=ALU.add, op1=ALU.add,
                        )
                        xb_cur = xb_new
                # xb_cur == x_3 (the odd-step fp32 carry)

                def pslice(bufs, tt, nslots):
                    b = bufs[tt // CH]
                    return b[:, ds((tt % CH) * BSH, nslots * BSH)]

                # ---- pair regime: t = 4..14 (bridges boot -> quads) ----
                for t in range(4, 16, 2):
                    s = t % CH
                    xr4 = pslice(xr_bufs, t - 4, 2)
                    u4 = pslice(ub_bufs, t - 4, 2)

                    psx = psxp.tile([N, 4 * BSH], F32, tag="psx")
                    if t >= 8:
                        u8 = pslice(ub_bufs, t - 8, 2)
                        nc.tensor.matmul(
                            psx[:, 0:2 * BSH], GATm[:], u4, start=True, stop=False
                        )
                        nc.tensor.matmul(
                            psx[:, 0:2 * BSH], GBTm[:], u8, start=False, stop=False
                        )
                        nc.tensor.matmul(
                            psx[:, 0:2 * BSH], A4Tm[:], xr4, start=False, stop=True
                        )
                    else:
                        nc.tensor.matmul(
                            psx[:, 0:2 * BSH], G4Tm[:], u4, start=True, stop=False
                        )
                        nc.tensor.matmul(
                            psx[:, 0:2 * BSH], A4Tm[:], xr4, start=False, stop=True
                        )
                    psy = psyp.tile([N, 4 * BSH], F32, tag="psy")
                    nc.tensor.matmul(
                        psy[:, 0:2 * BSH], Q4Tm[:], u4, start=True, stop=False
                    )
                    nc.tensor.matmul(
                        psy[:, 0:2 * BSH], P4Tm[:], xr4, start=False, stop=True
                    )

                    dxe = dxp.tile([N, BSH], F32, tag="dxe")
                    nc.scalar.copy(dxe[:], psx[:, 0:BSH])
                    nc.scalar.activation(
                        ub[:, ds(s * BSH, 2 * BSH)], psy[:, 0:2 * BSH], ACTF.Tanh,
                        bias=r4_c[:], scale=1.0,
                    )
                    nc.vector.scalar_tensor_tensor(
                        xrb[:, ds(s * BSH, BSH)], dxe[:], cc_c[:], xb_cur[:],
                        op0=ALU.add, op1=ALU.add,
                    )
                    s2 = dxp.tile([N, BSH], F32, tag="s2")
                    nc.vector.scalar_tensor_tensor(
                        s2[:], psx[:, ds(BSH, BSH)], cc2_c[:], dxe[:],
                        op0=ALU.add, op1=ALU.add,
                    )
                    nc.vector.tensor_add(
                        xrb[:, ds((s + 1) * BSH, BSH)], s2[:], xb_cur[:]
                    )
                    xb_new = xbp.tile([N, BSH], F32, tag="xb")
                    nc.gpsimd.tensor_add(xb_new[:], s2[:], xb_cur[:])
                    xb_cur = xb_new

                # ---- quad regime: t = 16, 20, ..., tmax-4 ----
                for t in range(16, tmax, 4):
                    s = t % CH
                    if s == 0:
                        cidx = t // CH
                        xrb = xrbp.tile([N, CH * BSH], mdt, tag="xrb")
                        ub = ubp.tile([N, CH * BSH], mdt, tag="ub")
                        xr_bufs[cidx] = xrb
                        ub_bufs[cidx] = ub
                        xr_bufs.pop(cidx - 2, None)
                        ub_bufs.pop(cidx - 2, None)

                    xr8 = pslice(xr_bufs, t - 8, 4)
                    u8q = pslice(ub_bufs, t - 8, 4)
                    u16q = pslice(ub_bufs, t - 16, 4)

                    # psx quad: column j = delta_{t+j-1}  (cc8 baked in via
                    # the rank-1 cc8 (x) ones matmul)
                    psx = psxp.tile([N, 4 * BSH], F32, tag="psx")
                    nc.tensor.matmul(psx[:], ccrow[:], ones4b[:],
                                     start=True, stop=False)
                    nc.tensor.matmul(psx[:], GA8Tm[:], u8q, start=False, stop=False)
                    nc.tensor.matmul(psx[:], GB8Tm[:], u16q, start=False, stop=False)
                    nc.tensor.matmul(psx[:], A8Tm[:], xr8, start=False, stop=True)
                    psy = psyp.tile([N, 4 * BSH], F32, tag="psy")
                    nc.tensor.matmul(psy[:], Q8Tm[:], u8q, start=True, stop=False)
                    nc.tensor.matmul(psy[:], P8Tm[:], xr8, start=False, stop=True)

                    # ACT: evacuate delta_t; tanh over all 4 steps at once
                    dxe = dxp.tile([N, BSH], F32, tag="dxe")
                    nc.scalar.copy(dxe[:], psx[:, 0:BSH])
                    nc.scalar.activation(
                        ub[:, ds(s * BSH, 4 * BSH)], psy[:], ACTF.Tanh,
                        bias=r8_c[:], scale=1.0,
                    )

                    # DVE: prefix sums (delta increments include cc8) + the
                    # full-quad sum via an off-chain strided reduce
                    s1 = dxp.tile([N, BSH], F32, tag="s1")
                    nc.vector.tensor_add(s1[:], psx[:, ds(BSH, BSH)], dxe[:])
                    s2q = dxp.tile([N, BSH], F32, tag="s2q")
                    nc.vector.tensor_add(s2q[:], psx[:, ds(2 * BSH, BSH)], s1[:])
                    sQ = dxp.tile([N, BSH], F32, tag="sQ")
                    nc.vector.tensor_reduce(
                        sQ[:], psx[:].rearrange("p (a b) -> p b a", a=4),
                        AXIS.X, ALU.add,
                    )
                    # fp16 state writes x_t..x_{t+3}
                    nc.vector.tensor_add(
                        xrb[:, ds(s * BSH, BSH)], dxe[:], xb_cur[:]
                    )
                    nc.vector.tensor_add(
                        xrb[:, ds((s + 1) * BSH, BSH)], s1[:], xb_cur[:]
                    )
                    nc.vector.tensor_add(
                        xrb[:, ds((s + 2) * BSH, BSH)], s2q[:], xb_cur[:]
                    )
                    # GPSIMD: fp32 quad carry first, then the last state
                    xb_new = xbp.tile([N, BSH], F32, tag="xb")
                    nc.gpsimd.tensor_add(xb_new[:], sQ[:], xb_cur[:])
                    nc.gpsimd.tensor_add(
                        xrb[:, ds((s + 3) * BSH, BSH)], sQ[:], xb_cur[:]
                    )
                    xb_cur = xb_new

                    if s + 4 == CH:
                        c = t // CH
                        nc.sync.dma_start(
                            out=out[:, ds(c * CH * BSH, CH * BSH)],
                            in_=xrb[:, 0:CH * BSH],
                        )

    nc.compile()
    return nc


_CACHED = {}


def _get_program(tmax=TMAX, tc_chunk=64, mdt=FP16):
    key = (tmax, tc_chunk, str(mdt))
    if key not in _CACHED:
        _CACHED[key] = build_program(tmax, tc_chunk, mdt)
    return _CACHED[key]


def make_in_maps(inputs, tmax=TMAX):
    X0 = np.ascontiguousarray(np.asarray(inputs["X0"], dtype=np.float32))
    base = {
        name: np.ascontiguousarray(np.asarray(inputs[name], dtype=np.float32))
        for name in PARAM_NAMES
    }
    base["bx"] = np.ascontiguousarray(
        np.asarray(inputs["bx"], dtype=np.float32).reshape(N, 1)
    )
    base["by"] = np.ascontiguousarray(
        np.asarray(inputs["by"], dtype=np.float32).reshape(N, 1)
    )
    in_maps = []
    for c in range(NCORES):
        m = dict(base)
        m["x0"] = np.ascontiguousarray(X0[c * BSH:(c + 1) * BSH].T)
        in_maps.append(m)
    return in_maps


def run_spmd(inputs, tmax=TMAX, tc_chunk=64, trace=False, tmpdir=None, mdt=FP16):
    nc = _get_program(tmax, tc_chunk, mdt)
    in_maps = make_in_maps(inputs, tmax)
    res = run_bass_kernel_spmd(
        nc, in_maps, list(range(NCORES)), trace=trace, tmpdir=tmpdir
    )
    X0 = np.asarray(inputs["X0"], dtype=np.float32)
    outs = []
    for c in range(NCORES):
        o = np.asarray(res.results[c]["out"])        # [N, tmax*BSH] fp16
        o = o.reshape(N, tmax, BSH).transpose(2, 1, 0).astype(np.float32)
        outs.append(o)                               # (BSH, tmax, N)
    full = np.concatenate(outs, axis=0)              # (BS, tmax, N)
    full[:, 0, :] = X0                               # exact t=0 plane
    return full, res


def kernel(**inputs):
    full, _ = run_spmd(inputs)
    return full


# revision 21
# speedup vs baseline: 1.2705x; 1.1465x over previous
"""LurieNet-k Trainium2 kernel (lag-4 paired recurrence, fp16 operands).

Computes, from the raw parametrization tensors, the matrices
  C = UC @ SC @ VC^T,  B = UB @ SB @ VB^T,
  A = 0.5*UA @ SA @ UA^T + 0.5*YA  (SA = -(alpha_upp*I + GA))
entirely on device (matrix exponentials of skew matrices via
scaling-and-squaring Taylor), then runs the 511-step recurrence
  u_t = tanh(C x_t + by);  x_{t+1} = x_t + 0.01*(A x_t + B u_t + bx)
on a (128, 64) state shard per NeuronCore (batch data-parallel over
the 8 cores).

Structure: the naive step is a serial tanh->matmul->tanh round trip
(~860ns on TRN2: ACT access latency + 2 sem hops + PE). Because the
tanh self-coupling Q = 0.01*C*B has tiny norm (~3e-4), the recurrence
is re-expanded to an (almost) exact LAG-4 form: every quantity at
step t is computed from state/tanh values at steps t-4/t-3 (and t-8
/t-7 for a first-order staleness extrapolation of the x-chain's u
terms, which kills the dominant scheme error: measured 1.2e-2 plain
-> 3.2e-4 extrapolated, fp16). All matmul inputs are then >= 2
pair-iterations old, so nothing serializes, and steps are processed
in PAIRS:
  - one 128-wide matmul per weight per pair (halves LDWEIGHTS, the
    PE throughput limit)
  - one 128-wide tanh per pair (halves ACT's per-instruction access
    latency tax)
  - PSUM evacuation split across ACT (Copy w/ bias for delta_t),
    DVE (pair-sum + the two fp16 state writes), and GPSIMD (fp32
    pair carry, SBUF-only since GPSIMD cannot read PSUM).
Weights/states/tanh values are fp16 (same PE speed as bf16, 8x finer
rounding); the fp32 carry keeps the state exact (all partial sums
fp32). Output is written fp16 time-major [n, t, b] straight from the
state buffer (no on-device transpose) and transposed to (b, t, n)
fp32 on the host during unsharding; the t=0 plane is restored
exactly from X0. Measured end-to-end rel err ~4e-4 (budget 2e-2).
"""

import sys

for _p in ("/opt/trn_rl_repo",):
    if _p not in sys.path:
        sys.path.insert(0, _p)

import numpy as np

import concourse.bass as bass
import concourse.mybir as mybir
import concourse.tile as tile
from concourse import bacc
from concourse import bass_isa
from concourse.bass import ds
from concourse.bass_utils import run_bass_kernel_spmd
from concourse.masks import make_identity, make_upper_triangular

F32 = mybir.dt.float32
F32R = mybir.dt.float32r
FP16 = mybir.dt.float16
ALU = mybir.AluOpType
ACTF = mybir.ActivationFunctionType
AXIS = mybir.AxisListType

N = 128          # state dim
TMAX = 512       # time steps (including t=0)
BS = 512         # global batch
NCORES = 8
BSH = BS // NCORES   # 64 batch columns per core
STEP = 0.01
KTOP = 4

EXPM_SCAL = 3    # expm scaling: X = S / 2**EXPM_SCAL, then 3 squarings
EXPM_TERMS = 4   # Taylor terms in the Horner evaluation

PARAM_NAMES = [
    "ZA_Y", "ZA_U", "ZA_G", "ZB_U", "ZB_V", "ZB_S", "ZC_U", "ZC_V", "ZC_S",
]


def build_program(tmax=TMAX, tc_chunk=64, mdt=FP16):
    """Build the single-NeuronCore Bass program (run SPMD on all 8 cores)."""
    assert tmax % tc_chunk == 0 and tc_chunk % 2 == 0
    nc = bacc.Bacc(
        "TRN2",
        target_bir_lowering=False,
        debug=False,
        enable_asserts=False,
        num_devices=NCORES,
    )

    x0 = nc.dram_tensor("x0", [N, BSH], F32, kind="ExternalInput")
    zs = {
        name: nc.dram_tensor(name, [N, N], F32, kind="ExternalInput")
        for name in PARAM_NAMES
    }
    bx_d = nc.dram_tensor("bx", [N, 1], F32, kind="ExternalInput")
    by_d = nc.dram_tensor("by", [N, 1], F32, kind="ExternalInput")
    # time-major fp16 output: out[n, t*BSH + b]; host transposes to (b,t,n)
    out = nc.dram_tensor("out", [N, tmax * BSH], FP16, kind="ExternalOutput")

    with tile.TileContext(nc) as tc:
        with tc.tile_pool(name="const", bufs=1) as constp:
            ident = constp.tile([N, N], F32, tag="ident")
            make_identity(nc, ident[:])
            masku = constp.tile([N, N], F32, tag="masku")
            make_upper_triangular(nc, masku[:], val=1.0, diag=False)
            ident_r32 = constp.tile([N, N], F32R, tag="ident_r32")
            nc.vector.tensor_copy(ident_r32[:], ident[:])

            by_c = constp.tile([N, 1], F32, tag="by")
            nc.sync.dma_start(out=by_c[:], in_=by_d[:])
            bx_c = constp.tile([N, 1], F32, tag="bxraw")
            nc.sync.dma_start(out=bx_c[:], in_=bx_d[:])
            bxp_c = constp.tile([N, 1], F32, tag="bxp")
            nc.vector.tensor_scalar_mul(bxp_c[:], bx_c[:], STEP)
            x0_c = constp.tile([N, BSH], F32, tag="x0c")
            nc.sync.dma_start(out=x0_c[:], in_=x0[:])

            # ------- runtime weights (transposed, fp16) -------
            P4Tm = constp.tile([N, N], mdt, tag="P4Tm")    # (C At^4)^T
            Q4Tm = constp.tile([N, N], mdt, tag="Q4Tm")    # (C S3 G)^T
            A4Tm = constp.tile([N, N], mdt, tag="A4Tm")    # (0.01A At^3)^T
            GATm = constp.tile([N, N], mdt, tag="GATm")    # extrap u_{t-4} w
            GBTm = constp.tile([N, N], mdt, tag="GBTm")    # extrap u_{t-8} w
            G4Tm = constp.tile([N, N], mdt, tag="G4Tm")    # plain (boot pairs)
            A01Tm = constp.tile([N, N], mdt, tag="A01Tm")  # (0.01 A)^T (boot)
            BpTm = constp.tile([N, N], mdt, tag="BpTm")    # (0.01 B)^T (boot)
            CTf32 = constp.tile([N, N], F32, tag="CTf32")  # C^T fp32 (boot)
            r4_c = constp.tile([N, 1], F32, tag="r4c")     # C S3 c + by
            cc_c = constp.tile([N, 1], F32, tag="ccc")     # delta const
            # quad (lag-8) weights
            P8Tm = constp.tile([N, N], mdt, tag="P8Tm")    # (C At^8)^T
            Q8Tm = constp.tile([N, N], mdt, tag="Q8Tm")    # (C S7 G)^T
            A8Tm = constp.tile([N, N], mdt, tag="A8Tm")    # (0.01A At^7)^T
            GA8Tm = constp.tile([N, N], mdt, tag="GA8Tm")  # extrap u_{t-8} w
            GB8Tm = constp.tile([N, N], mdt, tag="GB8Tm")  # extrap u_{t-16} w
            r8_c = constp.tile([N, 1], F32, tag="r8c")
            ccq1_c = constp.tile([N, 1], F32, tag="ccq1")  # cc8 multiples
            ccq2_c = constp.tile([N, 1], F32, tag="ccq2")
            ccq3_c = constp.tile([N, 1], F32, tag="ccq3")
            ccq4_c = constp.tile([N, 1], F32, tag="ccq4")

            with (
                tc.tile_pool(name="zbuf", bufs=1) as zp,
                tc.tile_pool(name="work", bufs=2) as wp,
                tc.tile_pool(name="eres", bufs=1) as ep,
                tc.tile_pool(name="small", bufs=1) as sp,
                tc.tile_pool(name="pss", bufs=4, space="PSUM") as psp,
            ):
                zt = {}
                # expm inputs first: their chains gate the whole setup
                load_order = ["ZC_U", "ZC_V", "ZB_U", "ZB_V", "ZA_U",
                              "ZC_S", "ZB_S", "ZA_G", "ZA_Y"]
                for name in load_order:
                    zt[name] = zp.tile([N, N], F32, tag=name, name=f"z_{name}")
                    nc.sync.dma_start(out=zt[name][:], in_=zs[name][:])

                def expm_batch(specs):
                    """Interleaved expm(skew(Z))^T for all matrices at once.

                    Maintains the (T, T^T) pair through Horner + squaring so
                    no PE transposes are needed: with negX = X^T = -X,
                      X @ T     = matmul(lhsT=negX, rhs=T)
                      T^T @ X^T = matmul(lhsT=T,    rhs=negX)
                    """
                    scal = 1.0 / (2.0 ** EXPM_SCAL)
                    negx = {}
                    t_cur = {}
                    tt_cur = {}
                    for z_tile, tag in specs:
                        us = wp.tile([N, N], F32R, tag="us_r", name=f"us_{tag}")
                        nc.vector.scalar_tensor_tensor(
                            us[:], z_tile[:], scal, masku[:],
                            op0=ALU.mult, op1=ALU.mult,
                        )
                        pst = psp.tile([N, N], F32R, tag="ps", bufs=7,
                                       name=f"pst_{tag}")
                        nc.tensor.transpose(pst[:], us[:], ident_r32[:])
                        nx = wp.tile([N, N], F32R, tag=f"negx_{tag}", bufs=1,
                                     name=f"negx_{tag}")
                        nc.vector.scalar_tensor_tensor(
                            nx[:], pst[:], 1.0, us[:],
                            op0=ALU.mult, op1=ALU.subtract,
                        )
                        negx[tag] = nx
                        t_cur[tag] = ident_r32
                        tt_cur[tag] = ident_r32
                    for j in range(EXPM_TERMS, 0, -1):
                        for _, tag in specs:
                            psa = psp.tile([N, N], F32, tag="ps", bufs=7)
                            nc.tensor.matmul(
                                psa[:], negx[tag][:], t_cur[tag][:],
                                start=True, stop=True,
                            )
                            t_new = wp.tile([N, N], F32R, tag=f"T_{tag}",
                                            bufs=2, name=f"T_{tag}")
                            nc.vector.scalar_tensor_tensor(
                                t_new[:], psa[:], 1.0 / j, ident_r32[:],
                                op0=ALU.mult, op1=ALU.add,
                            )
                            t_cur[tag] = t_new
                    for _, tag in specs:
                        pst = psp.tile([N, N], F32R, tag="ps", bufs=7,
                                       name=f"ptt_{tag}")
                        nc.tensor.transpose(pst[:], t_cur[tag][:], ident_r32[:])
                        tt_new = wp.tile([N, N], F32R, tag=f"TT_{tag}",
                                         bufs=2, name=f"TT_{tag}")
                        nc.scalar.copy(tt_new[:], pst[:])
                        tt_cur[tag] = tt_new
                    for _ in range(EXPM_SCAL):
                        for _, tag in specs:
                            psa = psp.tile([N, N], F32, tag="ps", bufs=7)
                            psb = psp.tile([N, N], F32, tag="ps", bufs=7)
                            nc.tensor.matmul(
                                psa[:], tt_cur[tag][:], t_cur[tag][:],
                                start=True, stop=True,
                            )
                            nc.tensor.matmul(
                                psb[:], t_cur[tag][:], tt_cur[tag][:],
                                start=True, stop=True,
                            )
                            t_new = wp.tile([N, N], F32R, tag=f"T_{tag}",
                                            bufs=2, name=f"T_{tag}")
                            tt_new = wp.tile([N, N], F32R, tag=f"TT_{tag}",
                                             bufs=2, name=f"TT_{tag}")
                            nc.scalar.copy(t_new[:], psa[:])
                            nc.scalar.copy(tt_new[:], psb[:])
                            t_cur[tag], tt_cur[tag] = t_new, tt_new
                    return tt_cur

                eres = expm_batch([
                    (zt["ZC_U"], "UCT"), (zt["ZC_V"], "VCT"),
                    (zt["ZB_U"], "UBT"), (zt["ZB_V"], "VBT"),
                    (zt["ZA_U"], "UAT"),
                ])
                uct, vct = eres["UCT"], eres["VCT"]
                ubt, vbt = eres["UBT"], eres["VBT"]
                uat = eres["UAT"]

                def absdiag_col(z_tile, tag):
                    tmp = wp.tile([N, N], F32, tag="us")
                    nc.vector.tensor_mul(tmp[:], z_tile[:], ident[:])
                    col = sp.tile([N, 1], F32, tag=tag, name=f"col_{tag}")
                    nc.vector.tensor_reduce(
                        col[:], tmp[:], AXIS.X, ALU.add, apply_absolute_value=True
                    )
                    return col

                dc_col = absdiag_col(zt["ZC_S"], "dc")
                db_col = absdiag_col(zt["ZB_S"], "db")
                ga_col = absdiag_col(zt["ZA_G"], "ga")

                # top-4: alpha = sqrt(sum_i (b_i c_i)^2), b/c sorted desc.
                bwork = sp.tile([N, 1], F32, tag="bwork")
                cwork = sp.tile([N, 1], F32, tag="cwork")
                nc.vector.tensor_copy(bwork[:], db_col[:])
                nc.vector.tensor_copy(cwork[:], dc_col[:])
                acc = sp.tile([N, 1], F32, tag="acc")
                nc.vector.memset(acc[:], 0.0)
                bmax = sp.tile([N, 1], F32, tag="bmax")
                cmax = sp.tile([N, 1], F32, tag="cmax")
                prod = sp.tile([N, 1], F32, tag="prod")
                gmask = sp.tile([N, 1], F32, tag="gmask")
                tdrop = sp.tile([N, 1], F32, tag="tdrop")
                for i in range(KTOP):
                    nc.gpsimd.partition_all_reduce(
                        bmax[:], bwork[:], N, bass_isa.ReduceOp.max
                    )
                    nc.gpsimd.partition_all_reduce(
                        cmax[:], cwork[:], N, bass_isa.ReduceOp.max
                    )
                    nc.vector.tensor_mul(prod[:], bmax[:], cmax[:])
                    nc.vector.tensor_mul(prod[:], prod[:], prod[:])
                    nc.vector.tensor_add(acc[:], acc[:], prod[:])
                    if i < KTOP - 1:
                        nc.vector.tensor_single_scalar(
                            gmask[:], bwork[:], bmax[:], ALU.is_ge
                        )
                        nc.vector.tensor_mul(tdrop[:], bwork[:], gmask[:])
                        nc.vector.tensor_sub(bwork[:], bwork[:], tdrop[:])
                        nc.vector.tensor_single_scalar(
                            gmask[:], cwork[:], cmax[:], ALU.is_ge
                        )
                        nc.vector.tensor_mul(tdrop[:], cwork[:], gmask[:])
                        nc.vector.tensor_sub(cwork[:], cwork[:], tdrop[:])
                alpha = sp.tile([N, 1], F32, tag="alpha")
                nc.scalar.activation(alpha[:], acc[:], ACTF.Sqrt)

                sa05 = sp.tile([N, 1], F32, tag="sa05")
                nc.vector.tensor_scalar(
                    sa05[:], ga_col[:], alpha[:], -0.5, op0=ALU.add, op1=ALU.mult
                )
                sb01 = sp.tile([N, 1], F32, tag="sb01")
                nc.vector.tensor_scalar_mul(sb01[:], db_col[:], STEP)

                # C^T = VC @ (SC @ UC^T)
                p1 = wp.tile([N, N], F32R, tag="us_r", name="p1")
                nc.vector.tensor_scalar_mul(p1[:], uct[:], dc_col[:])
                psa = psp.tile([N, N], F32, tag="ps", bufs=7)
                nc.tensor.matmul(psa[:], vct[:], p1[:], start=True, stop=True)
                nc.vector.tensor_copy(CTf32[:], psa[:])

                # G^T = (0.01 B)^T = VB @ (0.01 SB @ UB^T)
                p2 = wp.tile([N, N], F32R, tag="us_r", name="p2")
                nc.vector.tensor_scalar_mul(p2[:], ubt[:], sb01[:])
                psb = psp.tile([N, N], F32, tag="ps", bufs=7)
                nc.tensor.matmul(psb[:], vbt[:], p2[:], start=True, stop=True)
                nc.vector.tensor_copy(BpTm[:], psb[:])
                W1T = ep.tile([N, N], F32, tag="W1T")      # G^T fp32
                nc.scalar.copy(W1T[:], psb[:])
                # untransposed G = 0.01 B = UB @ (0.01 SB @ VB^T)
                p2b = wp.tile([N, N], F32R, tag="us_r", name="p2b")
                nc.vector.tensor_scalar_mul(p2b[:], vbt[:], sb01[:])
                psb2 = psp.tile([N, N], F32, tag="ps", bufs=7)
                nc.tensor.matmul(psb2[:], ubt[:], p2b[:], start=True, stop=True)
                bp_un = ep.tile([N, N], F32, tag="Bpun")
                nc.vector.tensor_copy(bp_un[:], psb2[:])

                # A: M = UA @ (sa05 * UA^T); YA part via masked transpose
                p3 = wp.tile([N, N], F32R, tag="us_r", name="p3")
                nc.vector.tensor_scalar_mul(p3[:], uat[:], sa05[:])
                psm = psp.tile([N, N], F32, tag="ps", bufs=7)
                nc.tensor.matmul(psm[:], uat[:], p3[:], start=True, stop=True)
                uy = wp.tile([N, N], F32, tag="us")
                nc.vector.tensor_mul(uy[:], zt["ZA_Y"][:], masku[:])
                pst2 = psp.tile([N, N], F32, tag="ps", bufs=7)
                nc.tensor.transpose(pst2[:], uy[:], ident[:])
                nc.vector.tensor_scalar_mul(uy[:], uy[:], 0.5 * STEP)
                q2 = wp.tile([N, N], F32, tag="T")
                nc.vector.scalar_tensor_tensor(
                    q2[:], pst2[:], 0.5 * STEP, uy[:], op0=ALU.mult, op1=ALU.subtract
                )
                # (0.01 A)^T fp32 + fp16; untransposed 0.01 A fp32
                A01Tf = ep.tile([N, N], F32, tag="A01Tf")
                nc.vector.scalar_tensor_tensor(
                    A01Tf[:], psm[:], STEP, q2[:], op0=ALU.mult, op1=ALU.add
                )
                nc.vector.tensor_copy(A01Tm[:], A01Tf[:])
                a01_un = ep.tile([N, N], F32, tag="A01un")
                nc.vector.scalar_tensor_tensor(
                    a01_un[:], psm[:], STEP, q2[:], op0=ALU.mult, op1=ALU.subtract
                )

                def emit_mm(x_tile, tagname):
                    ps = psp.tile([N, N], F32, tag="ps", bufs=7, name=f"ps_{tagname}")
                    nc.tensor.matmul(ps[:], a01_un[:], x_tile[:], start=True, stop=True)
                    return ps

                def emit_gmm(x_tile, tagname):
                    ps = psp.tile([N, N], F32, tag="ps", bufs=7, name=f"pg_{tagname}")
                    nc.tensor.matmul(ps[:], bp_un[:], x_tile[:], start=True, stop=True)
                    return ps

                def fin_at(ps, x_tile, out_tile):
                    """out = x + psum  (the At^T multiply-add tail)."""
                    nc.vector.scalar_tensor_tensor(
                        out_tile[:], ps[:], 1.0, x_tile[:], op0=ALU.mult, op1=ALU.add
                    )

                # Chains T_k = (At^T)^k C^T and U_k = (At^T)^k (0.01A)^T are
                # independent; emit the waves interleaved so neither chain
                # head-blocks the in-order PE queue on the other's DVE tail.
                T1 = ep.tile([N, N], F32, tag="T1")
                U1 = ep.tile([N, N], F32, tag="U1")
                W2T = ep.tile([N, N], F32, tag="W2T")
                ps_t1 = emit_mm(CTf32, "T1")
                ps_u1 = emit_mm(A01Tf, "U1")
                ps_w2 = emit_gmm(A01Tf, "W2")
                fin_at(ps_t1, CTf32, T1)
                fin_at(ps_u1, A01Tf, U1)
                nc.vector.tensor_copy(W2T[:], ps_w2[:])

                T2 = ep.tile([N, N], F32, tag="T2")
                U2 = ep.tile([N, N], F32, tag="U2")
                W3T = ep.tile([N, N], F32, tag="W3T")
                ps_t2 = emit_mm(T1, "T2")
                ps_u2 = emit_mm(U1, "U2")
                ps_w3 = emit_gmm(U1, "W3")
                fin_at(ps_t2, T1, T2)
                fin_at(ps_u2, U1, U2)
                nc.vector.tensor_copy(W3T[:], ps_w3[:])

                T3 = ep.tile([N, N], F32, tag="T3")
                U3 = ep.tile([N, N], F32, tag="U3")
                W4T = ep.tile([N, N], F32, tag="W4T")
                ps_t3 = emit_mm(T2, "T3")
                ps_u3 = emit_mm(U2, "U3")
                ps_w4 = emit_gmm(U2, "W4")
                fin_at(ps_t3, T2, T3)
                fin_at(ps_u3, U2, U3)
                nc.vector.tensor_copy(A4Tm[:], U3[:])
                nc.vector.tensor_copy(W4T[:], ps_w4[:])

                T4 = ep.tile([N, N], F32, tag="T4")
                U4 = ep.tile([N, N], F32, tag="U4")
                W5T = ep.tile([N, N], F32, tag="W5T")
                ssum = wp.tile([N, N], F32, tag="us")
                nc.vector.tensor_add(ssum[:], CTf32[:], T1[:])
                nc.vector.tensor_add(ssum[:], ssum[:], T2[:])
                nc.vector.tensor_add(ssum[:], ssum[:], T3[:])
                ps_t4 = emit_mm(T3, "T4")
                ps_q4 = emit_gmm(ssum, "Q4")
                ps_u4 = emit_mm(U3, "U4")
                ps_w5 = emit_gmm(U3, "W5")
                fin_at(ps_t4, T3, T4)
                nc.vector.tensor_copy(P4Tm[:], T4[:])
                nc.vector.tensor_copy(Q4Tm[:], ps_q4[:])
                fin_at(ps_u4, U3, U4)
                nc.vector.tensor_copy(W5T[:], ps_w5[:])

                # extend the chains to At^8 for the quad (lag-8) regime
                T5 = ep.tile([N, N], F32, tag="T5")
                U5 = ep.tile([N, N], F32, tag="U5")
                W6T = ep.tile([N, N], F32, tag="W6T")
                ps_t5 = emit_mm(T4, "T5")
                ps_u5 = emit_mm(U4, "U5")
                ps_w6 = emit_gmm(U4, "W6")
                fin_at(ps_t5, T4, T5)
                fin_at(ps_u5, U4, U5)
                nc.vector.tensor_copy(W6T[:], ps_w6[:])

                T6 = ep.tile([N, N], F32, tag="T6")
                U6 = ep.tile([N, N], F32, tag="U6")
                W7T = ep.tile([N, N], F32, tag="W7T")
                ps_t6 = emit_mm(T5, "T6")
                ps_u6 = emit_mm(U5, "U6")
                ps_w7 = emit_gmm(U5, "W7")
                fin_at(ps_t6, T5, T6)
                fin_at(ps_u6, U5, U6)
                nc.vector.tensor_copy(W7T[:], ps_w7[:])

                T7 = ep.tile([N, N], F32, tag="T7")
                U7 = ep.tile([N, N], F32, tag="U7")
                W8T = ep.tile([N, N], F32, tag="W8T")
                ps_t7 = emit_mm(T6, "T7")
                ps_u7 = emit_mm(U6, "U7")
                ps_w8 = emit_gmm(U6, "W8")
                fin_at(ps_t7, T6, T7)
                fin_at(ps_u7, U6, U7)
                nc.vector.tensor_copy(A8Tm[:], U7[:])
                nc.vector.tensor_copy(W8T[:], ps_w8[:])

                T8 = ep.tile([N, N], F32, tag="T8")
                nc.vector.tensor_add(ssum[:], ssum[:], T4[:])
                nc.vector.tensor_add(ssum[:], ssum[:], T5[:])
                nc.vector.tensor_add(ssum[:], ssum[:], T6[:])
                nc.vector.tensor_add(ssum[:], ssum[:], T7[:])
                ps_t8 = emit_mm(T7, "T8")
                ps_q8 = emit_gmm(ssum, "Q8")
                fin_at(ps_t8, T7, T8)
                nc.vector.tensor_copy(P8Tm[:], T8[:])
                nc.vector.tensor_copy(Q8Tm[:], ps_q8[:])

                # GA = sum_j W_j*(1+(4-j)/4); GB = -sum_j W_j*(4-j)/4
                # G4 (plain) = sum_j W_j
                acc1 = wp.tile([N, N], F32, tag="us")
                nc.vector.tensor_scalar_mul(acc1[:], W1T[:], 1.75)
                nc.vector.scalar_tensor_tensor(
                    acc1[:], W2T[:], 1.5, acc1[:], op0=ALU.mult, op1=ALU.add
                )
                nc.vector.scalar_tensor_tensor(
                    acc1[:], W3T[:], 1.25, acc1[:], op0=ALU.mult, op1=ALU.add
                )
                nc.vector.scalar_tensor_tensor(
                    acc1[:], W4T[:], 1.0, acc1[:], op0=ALU.mult, op1=ALU.add
                )
                nc.vector.tensor_copy(GATm[:], acc1[:])
                acc2 = wp.tile([N, N], F32, tag="T")
                nc.vector.tensor_scalar_mul(acc2[:], W1T[:], -0.75)
                nc.vector.scalar_tensor_tensor(
                    acc2[:], W2T[:], -0.5, acc2[:], op0=ALU.mult, op1=ALU.add
                )
                nc.vector.scalar_tensor_tensor(
                    acc2[:], W3T[:], -0.25, acc2[:], op0=ALU.mult, op1=ALU.add
                )
                nc.vector.tensor_copy(GBTm[:], acc2[:])
                acc3 = wp.tile([N, N], F32, tag="us", name="acc3")
                nc.vector.tensor_add(acc3[:], W1T[:], W2T[:])
                nc.vector.tensor_add(acc3[:], acc3[:], W3T[:])
                nc.vector.tensor_add(acc3[:], acc3[:], W4T[:])
                nc.vector.tensor_copy(G4Tm[:], acc3[:])

                # GA8 = sum_j W_j*(1+(8-j)/8); GB8 = -sum_j W_j*(8-j)/8
                WTs = [W1T, W2T, W3T, W4T, W5T, W6T, W7T, W8T]
                acc8a = wp.tile([N, N], F32, tag="us", name="acc8a")
                nc.vector.tensor_scalar_mul(acc8a[:], W1T[:], 1.875)
                for j in range(2, 9):
                    nc.vector.scalar_tensor_tensor(
                        acc8a[:], WTs[j - 1][:], 1.0 + (8.0 - j) / 8.0, acc8a[:],
                        op0=ALU.mult, op1=ALU.add,
                    )
                nc.vector.tensor_copy(GA8Tm[:], acc8a[:])
                acc8b = wp.tile([N, N], F32, tag="T", name="acc8b")
                nc.vector.tensor_scalar_mul(acc8b[:], W1T[:], -0.875)
                for j in range(2, 8):
                    nc.vector.scalar_tensor_tensor(
                        acc8b[:], WTs[j - 1][:], -(8.0 - j) / 8.0, acc8b[:],
                        op0=ALU.mult, op1=ALU.add,
                    )
                nc.vector.tensor_copy(GB8Tm[:], acc8b[:])

                # vectors: c = 0.01 bx; w1 = At c, w2 = At w1, w3 = At w2
                # r4 = C (c+w1+w2+w3) + by
                # cc = c + 0.01A (c+w1+w2)
                def atv(v_in, tagname):
                    ps = psp.tile([N, 1], F32, tag="ps", bufs=7, name=f"pv_{tagname}")
                    nc.tensor.matmul(ps[:], A01Tf[:], v_in[:], start=True, stop=True)
                    v_out = sp.tile([N, 1], F32, tag=tagname)
                    nc.vector.scalar_tensor_tensor(
                        v_out[:], ps[:], 1.0, v_in[:], op0=ALU.mult, op1=ALU.add
                    )
                    return v_out

                w1 = atv(bxp_c, "w1")
                w2 = atv(w1, "w2")
                w3 = atv(w2, "w3")
                w4 = atv(w3, "w4")
                w5 = atv(w4, "w5")
                w6 = atv(w5, "w6")
                w7 = atv(w6, "w7")
                vs = sp.tile([N, 1], F32, tag="vs")
                nc.vector.tensor_add(vs[:], bxp_c[:], w1[:])
                vs2 = sp.tile([N, 1], F32, tag="vs2")
                nc.vector.tensor_add(vs2[:], vs[:], w2[:])
                vs3 = sp.tile([N, 1], F32, tag="vs3")
                nc.vector.tensor_add(vs3[:], vs2[:], w3[:])
                psr4 = psp.tile([N, 1], F32, tag="ps", bufs=7, name="psr4")
                nc.tensor.matmul(psr4[:], CTf32[:], vs3[:], start=True, stop=True)
                nc.vector.scalar_tensor_tensor(
                    r4_c[:], psr4[:], 1.0, by_c[:], op0=ALU.mult, op1=ALU.add
                )
                pscc = psp.tile([N, 1], F32, tag="ps", bufs=7, name="pscc")
                nc.tensor.matmul(pscc[:], A01Tf[:], vs2[:], start=True, stop=True)
                nc.vector.scalar_tensor_tensor(
                    cc_c[:], pscc[:], 1.0, bxp_c[:], op0=ALU.mult, op1=ALU.add
                )
                cc2_c = constp.tile([N, 1], F32, tag="cc2c")
                nc.vector.tensor_scalar_mul(cc2_c[:], cc_c[:], 2.0)

                # lag-8 constants: r8 = C (sum_{k=0..7} At^k) c + by,
                # cc8 = c + 0.01A (sum_{k=0..6} At^k) c, exposed as a row
                # for the rank-1 (cc8 x ones) matmul into the quad psx.
                vs6 = sp.tile([N, 1], F32, tag="vs6")
                nc.vector.tensor_add(vs6[:], vs3[:], w4[:])
                nc.vector.tensor_add(vs6[:], vs6[:], w5[:])
                nc.vector.tensor_add(vs6[:], vs6[:], w6[:])
                vs7 = sp.tile([N, 1], F32, tag="vs7")
                nc.vector.tensor_add(vs7[:], vs6[:], w7[:])
                psr8 = psp.tile([N, 1], F32, tag="ps", bufs=7, name="psr8")
                nc.tensor.matmul(psr8[:], CTf32[:], vs7[:], start=True, stop=True)
                nc.vector.scalar_tensor_tensor(
                    r8_c[:], psr8[:], 1.0, by_c[:], op0=ALU.mult, op1=ALU.add
                )
                pscc8 = psp.tile([N, 1], F32, tag="ps", bufs=7, name="pscc8")
                nc.tensor.matmul(pscc8[:], A01Tf[:], vs6[:], start=True, stop=True)
                nc.vector.scalar_tensor_tensor(
                    ccq1_c[:], pscc8[:], 1.0, bxp_c[:], op0=ALU.mult, op1=ALU.add
                )
                nc.vector.tensor_scalar_mul(ccq2_c[:], ccq1_c[:], 2.0)
                nc.vector.tensor_scalar_mul(ccq3_c[:], ccq1_c[:], 3.0)
                nc.vector.tensor_scalar_mul(ccq4_c[:], ccq1_c[:], 4.0)

            # ------- recurrence: lag-4, paired steps -------
            with (
                tc.tile_pool(name="xrb", bufs=2) as xrbp,
                tc.tile_pool(name="ub", bufs=2) as ubp,
                tc.tile_pool(name="xb", bufs=4) as xbp,
                tc.tile_pool(name="dx", bufs=3) as dxp,
                tc.tile_pool(name="psy", bufs=3, space="PSUM") as psyp,
                tc.tile_pool(name="psx", bufs=3, space="PSUM") as psxp,
            ):
                CH = tc_chunk
                nchunks = tmax // CH

                xrb = xrbp.tile([N, CH * BSH], mdt, tag="xrb")
                ub = ubp.tile([N, CH * BSH], mdt, tag="ub")
                xr_bufs = {0: xrb}
                ub_bufs = {0: ub}

                # ---- bootstrap steps 0..3 (exact per-step form) ----
                nc.vector.tensor_copy(xrb[:, 0:BSH], x0_c[:])
                xb_cur = x0_c
                for k in range(4):
                    psyb = psyp.tile([N, 4 * BSH], F32, tag="psy", name=f"psyb{k}")
                    nc.tensor.matmul(
                        psyb[:, 0:BSH], CTf32[:], xb_cur[:], start=True, stop=True
                    )
                    nc.scalar.activation(
                        ub[:, ds(k * BSH, BSH)], psyb[:, 0:BSH], ACTF.Tanh,
                        bias=by_c[:], scale=1.0,
                    )
                    if k < 3:
                        psxb = psxp.tile([N, 4 * BSH], F32, tag="psx",
                                         name=f"psxb{k}")
                        nc.tensor.matmul(
                            psxb[:, 0:BSH], A01Tm[:], xrb[:, ds(k * BSH, BSH)],
                            start=True, stop=False,
                        )
                        nc.tensor.matmul(
                            psxb[:, 0:BSH], BpTm[:], ub[:, ds(k * BSH, BSH)],
                            start=False, stop=True,
                        )
                        xb_new = xbp.tile([N, BSH], F32, tag="xb", name=f"xbb{k}")
                        nc.vector.scalar_tensor_tensor(
                            xb_new[:], psxb[:, 0:BSH], bxp_c[:], xb_cur[:],
                            op0=ALU.add, op1=ALU.add,
                        )
                        nc.vector.scalar_tensor_tensor(
                            xrb[:, ds((k + 1) * BSH, BSH)], psxb[:, 0:BSH],
                            bxp_c[:], xb_cur[:], op0=ALU.add, op1=ALU.add,
                        )
                        xb_cur = xb_new
                # xb_cur == x_3 (the odd-step fp32 carry)

                def pslice(bufs, tt, nslots):
                    b = bufs[tt // CH]
                    return b[:, ds((tt % CH) * BSH, nslots * BSH)]

                # ---- pair regime: t = 4..14 (bridges boot -> quads) ----
                for t in range(4, 16, 2):
                    s = t % CH
                    xr4 = pslice(xr_bufs, t - 4, 2)
                    u4 = pslice(ub_bufs, t - 4, 2)

                    psx = psxp.tile([N, 4 * BSH], F32, tag="psx")
                    if t >= 8:
                        u8 = pslice(ub_bufs, t - 8, 2)
                        nc.tensor.matmul(
                            psx[:, 0:2 * BSH], GATm[:], u4, start=True, stop=False
                        )
                        nc.tensor.matmul(
                            psx[:, 0:2 * BSH], GBTm[:], u8, start=False, stop=False
                        )
                        nc.tensor.matmul(
                            psx[:, 0:2 * BSH], A4Tm[:], xr4, start=False, stop=True
                        )
                    else:
                        nc.tensor.matmul(
                            psx[:, 0:2 * BSH], G4Tm[:], u4, start=True, stop=False
                        )
                        nc.tensor.matmul(
                            psx[:, 0:2 * BSH], A4Tm[:], xr4, start=False, stop=True
                        )
                    psy = psyp.tile([N, 4 * BSH], F32, tag="psy")
                    nc.tensor.matmul(
                        psy[:, 0:2 * BSH], Q4Tm[:], u4, start=True, stop=False
                    )
                    nc.tensor.matmul(
                        psy[:, 0:2 * BSH], P4Tm[:], xr4, start=False, stop=True
                    )

                    dxe = dxp.tile([N, BSH], F32, tag="dxe")
                    nc.scalar.copy(dxe[:], psx[:, 0:BSH])
                    nc.scalar.activation(
                        ub[:, ds(s * BSH, 2 * BSH)], psy[:, 0:2 * BSH], ACTF.Tanh,
                        bias=r4_c[:], scale=1.0,
                    )
                    nc.vector.scalar_tensor_tensor(
                        xrb[:, ds(s * BSH, BSH)], dxe[:], cc_c[:], xb_cur[:],
                        op0=ALU.add, op1=ALU.add,
                    )
                    s2 = dxp.tile([N, BSH], F32, tag="s2")
                    nc.vector.scalar_tensor_tensor(
                        s2[:], psx[:, ds(BSH, BSH)], cc2_c[:], dxe[:],
                        op0=ALU.add, op1=ALU.add,
                    )
                    nc.vector.tensor_add(
                        xrb[:, ds((s + 1) * BSH, BSH)], s2[:], xb_cur[:]
                    )
                    xb_new = xbp.tile([N, BSH], F32, tag="xb")
                    nc.gpsimd.tensor_add(xb_new[:], s2[:], xb_cur[:])
                    xb_cur = xb_new

                # ---- quad regime: t = 16, 20, ..., tmax-4 ----
                for t in range(16, tmax, 4):
                    s = t % CH
                    if s == 0:
                        cidx = t // CH
                        xrb = xrbp.tile([N, CH * BSH], mdt, tag="xrb")
                        ub = ubp.tile([N, CH * BSH], mdt, tag="ub")
                        xr_bufs[cidx] = xrb
                        ub_bufs[cidx] = ub
                        xr_bufs.pop(cidx - 2, None)
                        ub_bufs.pop(cidx - 2, None)

                    xr8 = pslice(xr_bufs, t - 8, 4)
                    u8q = pslice(ub_bufs, t - 8, 4)
                    u16q = pslice(ub_bufs, t - 16, 4)

                    # psx quad: column j = delta_{t+j-1}  (cc8 baked in via
                    # the rank-1 cc8 (x) ones matmul)
                    psx = psxp.tile([N, 4 * BSH], F32, tag="psx")
                    nc.tensor.matmul(psx[:], GA8Tm[:], u8q, start=True, stop=False)
                    nc.tensor.matmul(psx[:], GB8Tm[:], u16q, start=False, stop=False)
                    nc.tensor.matmul(psx[:], A8Tm[:], xr8, start=False, stop=True)
                    psy = psyp.tile([N, 4 * BSH], F32, tag="psy")
                    nc.tensor.matmul(psy[:], Q8Tm[:], u8q, start=True, stop=False)
                    nc.tensor.matmul(psy[:], P8Tm[:], xr8, start=False, stop=True)

                    # ACT: evacuate delta_t; tanh over all 4 steps at once
                    dxe = dxp.tile([N, BSH], F32, tag="dxe")
                    nc.scalar.copy(dxe[:], psx[:, 0:BSH])
                    nc.scalar.activation(
                        ub[:, ds(s * BSH, 4 * BSH)], psy[:], ACTF.Tanh,
                        bias=r8_c[:], scale=1.0,
                    )

                    # DVE: quad sum via off-chain strided reduce; prefixes
                    # carry the cc8 increments so GPSIMD (which supports
                    # only plain tensor-tensor ops) can finish the states.
                    sQ = dxp.tile([N, BSH], F32, tag="sQ")
                    nc.vector.tensor_reduce(
                        sQ[:], psx[:].rearrange("p (a b) -> p b a", a=4),
                        AXIS.X, ALU.add,
                    )
                    xb_new = xbp.tile([N, BSH], F32, tag="xb")
                    nc.vector.scalar_tensor_tensor(
                        xb_new[:], sQ[:], ccq4_c[:], xb_cur[:],
                        op0=ALU.add, op1=ALU.add,
                    )
                    s1 = dxp.tile([N, BSH], F32, tag="s1")
                    nc.vector.scalar_tensor_tensor(
                        s1[:], psx[:, ds(BSH, BSH)], ccq2_c[:], dxe[:],
                        op0=ALU.add, op1=ALU.add,
                    )
                    s2q = dxp.tile([N, BSH], F32, tag="s2q")
                    nc.vector.scalar_tensor_tensor(
                        s2q[:], psx[:, ds(2 * BSH, BSH)], ccq1_c[:], s1[:],
                        op0=ALU.add, op1=ALU.add,
                    )
                    nc.vector.scalar_tensor_tensor(
                        xrb[:, ds(s * BSH, BSH)], dxe[:], ccq1_c[:], xb_cur[:],
                        op0=ALU.add, op1=ALU.add,
                    )
                    # GPSIMD: plain adds/copy only (s1/s2q are pre-biased
                    # with 2cc/3cc; xb_new already holds x_{t+3} fp32)
                    nc.gpsimd.tensor_copy(
                        xrb[:, ds((s + 3) * BSH, BSH)], xb_new[:]
                    )
                    nc.gpsimd.tensor_add(
                        xrb[:, ds((s + 1) * BSH, BSH)], s1[:], xb_cur[:]
                    )
                    nc.gpsimd.tensor_add(
                        xrb[:, ds((s + 2) * BSH, BSH)], s2q[:], xb_cur[:]
                    )
                    xb_cur = xb_new

                    if s + 4 == CH:
                        c = t // CH
                        nc.sync.dma_start(
                            out=out[:, ds(c * CH * BSH, CH * BSH)],
                            in_=xrb[:, 0:CH * BSH],
                        )

    nc.compile()
    return nc


_CACHED = {}


def _get_program(tmax=TMAX, tc_chunk=64, mdt=FP16):
    key = (tmax, tc_chunk, str(mdt))
    if key not in _CACHED:
        _CACHED[key] = build_program(tmax, tc_chunk, mdt)
    return _CACHED[key]


def make_in_maps(inputs, tmax=TMAX):
    X0 = np.ascontiguousarray(np.asarray(inputs["X0"], dtype=np.float32))
    base = {
        name: np.ascontiguousarray(np.asarray(inputs[name], dtype=np.float32))
        for name in PARAM_NAMES
    }
    base["bx"] = np.ascontiguousarray(
        np.asarray(inputs["bx"], dtype=np.float32).reshape(N, 1)
    )
    base["by"] = np.ascontiguousarray(
        np.asarray(inputs["by"], dtype=np.float32).reshape(N, 1)
    )
    in_maps = []
    for c in range(NCORES):
        m = dict(base)
        m["x0"] = np.ascontiguousarray(X0[c * BSH:(c + 1) * BSH].T)
        in_maps.append(m)
    return in_maps


def run_spmd(inputs, tmax=TMAX, tc_chunk=64, trace=False, tmpdir=None, mdt=FP16):
    nc = _get_program(tmax, tc_chunk, mdt)
    in_maps = make_in_maps(inputs, tmax)
    res = run_bass_kernel_spmd(
        nc, in_maps, list(range(NCORES)), trace=trace, tmpdir=tmpdir
    )
    X0 = np.asarray(inputs["X0"], dtype=np.float32)
    outs = []
    for c in range(NCORES):
        o = np.asarray(res.results[c]["out"])        # [N, tmax*BSH] fp16
        o = o.reshape(N, tmax, BSH).transpose(2, 1, 0).astype(np.float32)
        outs.append(o)                               # (BSH, tmax, N)
    full = np.concatenate(outs, axis=0)              # (BS, tmax, N)
    full[:, 0, :] = X0                               # exact t=0 plane
    return full, res


def kernel(**inputs):
    full, _ = run_spmd(inputs)
    return full
